# revision 1
# baseline (speedup 1.0000x reference)
"""Trainium2 Bass kernel for multi-head causal attention.

Problem: B=2, S=2048, D=1024, H=16, DH=64 (fp32), causal attention with
QKV projections and output projection summed over heads.

Sharding: 8 cores = (batch b in {0,1}) x (head-group hg in {0..3}, 4 heads
each).  Each core computes a partial output sum over its 4 heads for its
batch; the host sums the 4 partials per batch and adds b_O.

Device-side layout choices:
  - x inputs are transposed on the HOST to [D, S] so every projection matmul
    has its contraction dim (d) on partitions with zero on-device transposes.
  - q/k are produced directly in transposed layout qT/kT [e, s] (e on
    partitions), q pre-scaled by 1/ATTN_SCALE.
  - scores are computed transposed: S^T[j, i] (keys on partitions), exp is
    applied with no max subtraction (|scores| <= ~4 here, exp is safe), the
    causal mask is applied by skipping/zeroing masked regions.
  - PV uses v in natural layout [j, e] augmented with 64 ones columns, so the
    softmax denominator l[i] falls out of the same matmul broadcast across
    PSUM partitions 64..127 (reciprocal + multiply normalizes, no extra
    broadcast step).
  - out projection: lhsT = zT chunks, rhs = W_O, accumulated over e-chunks.

Dtypes: x inputs and W_Q/K/V are shipped as fp16 (halves the dominant DMA
traffic).  The attention operands (qT/kT/v/pt) are fp16 too — unlike f32r,
fp16 matmuls run 1 cycle/row at ANY free-dim width, so every score/PV
matmul is trimmed to the exact 128-aligned causal boundary and the
fully-masked columns are never computed, zeroed, or read.  zT and W_O stay
float32r; all matmul accumulation is fp32 PSUM.  Measured end-to-end
relative error vs the fp32 reference: 3.05e-4.

A BIR post-processing patch (installed on import) hoists excess sync waits
off instructions into standalone EventSemaphore ops — walrus codegen allows
only 1 wait on the fused 4-byte-weight-load matmul encoding and few on
other opcodes, and Tile emits more.
"""

import sys

import numpy as np

for _p in ("/opt/trn_rl_repo",):
    if _p not in sys.path:
        sys.path.insert(0, _p)

import concourse.bass as bass
import concourse.tile as tile
from concourse import mybir
from concourse.bass_utils import run_bass_kernel_spmd


def _hoist_matmul_waits(bir_json: bytes) -> bytes:
    """Move extra sync waits off Matmult instructions.

    The fused 4-byte-weight-load matmul encoding (fp32/f32r) only has room
    for one sync wait command in walrus codegen ("Too many sync wait
    commands").  Hoist all but one wait into standalone EventSemaphore
    instructions on the same engine queue immediately before the matmul —
    semantically identical (the sequencer blocks on them in order).
    """
    import orjson

    m = orjson.loads(bir_json)
    changed = False
    for fn in m.get("functions", []):
        for bb in fn.get("blocks", []):
            insts = bb.get("instructions", [])
            out = []
            for inst in insts:
                if True:
                    si = inst.get("sync_info") or {}
                    waits = si.get("on_wait") or []
                    if len(waits) > 1:
                        keep = waits[-1]
                        for wi, w in enumerate(waits[:-1]):
                            out.append({
                                "debug": inst.get("debug", 0),
                                "engine": inst["engine"],
                                "ins": [],
                                "name": f"{inst['name']}-hw{wi}",
                                "opcode": "EventSemaphore",
                                "outs": [],
                                "sync_info": {"on_update": [],
                                              "on_wait": [w]},
                            })
                        si["on_wait"] = [keep]
                        inst["sync_info"] = si
                        changed = True
                out.append(inst)
            bb["instructions"] = out
    if not changed:
        return bir_json
    return orjson.dumps(m)


def _install_bir_patch():
    from concourse import bass2jax as _b2j
    from concourse import bass_utils as _bu

    if getattr(_b2j, "_mm_wait_patch", False):
        return

    _orig = _bu.compile_bir_kernel

    def _patched(bir_json, tmpdir, neff_name="file.neff"):
        return _orig(_hoist_matmul_waits(bir_json), tmpdir, neff_name)

    _b2j.compile_bir_kernel = _patched
    _bu.compile_bir_kernel = _patched
    _b2j._mm_wait_patch = True


_install_bir_patch()

# Problem dims (hardcoded per harness contract).
B, S, D, H, DH = 2, 2048, 1024, 16, 64
ATTN_SCALE = 8.0
NCORES = 8
HL = H // (NCORES // B)  # 4 local heads per core
E = HL * DH              # 256 local head dims
P = 128
DC = D // P              # 8 contraction chunks
EC = E // P              # 2 e-chunks
NSB = S // P             # 16 s-blocks of 128
NI = 1024                # i-group width for score strips
NG = S // NI             # 2 i-groups
F32 = mybir.dt.float32
F32R = mybir.dt.float32r
F16 = mybir.dt.float16
AF = mybir.ActivationFunctionType


def _round_f32r(arr):
    """Round an fp32 array to float32r (tfloat32) representable values."""
    from neuronxcc.starfish.support import dtype as nxd
    a = np.ascontiguousarray(np.asarray(arr, dtype=np.float32))
    return np.asarray(nxd.static_cast(a, dtype=nxd.float32r)).view(np.float32)


def _emit(ctx, tc, xq, xk, xv, wq, wk, wv, wo, bq, bk, bv, masks, out):
    nc = tc.nc

    persist = ctx.enter_context(tc.tile_pool(name="persist", bufs=1))
    xstage = ctx.enter_context(tc.tile_pool(name="xstage", bufs=4))
    xvstage = ctx.enter_context(tc.tile_pool(name="xvstage", bufs=3))
    ptpool = ctx.enter_context(tc.tile_pool(name="ptp", bufs=8))
    outpool = ctx.enter_context(tc.tile_pool(name="outp", bufs=4))
    small = ctx.enter_context(tc.tile_pool(name="small", bufs=6))
    # PSUM budget (8 banks of [128, 2KB]):
    #   ps_s: score strips [128, 1024] = 2 banks x 2 bufs = 4
    #   ps_mm: proj / outproj [128, <=512] = 1 bank x 2 bufs = 2
    #   ps_z: PV accumulators [128, 512] = 1 bank x 2 bufs = 2
    ps_s = ctx.enter_context(tc.tile_pool(name="ps_s", bufs=2, space="PSUM"))
    ps_mm = ctx.enter_context(tc.tile_pool(name="ps_mm", bufs=2, space="PSUM"))
    ps_z = ctx.enter_context(tc.tile_pool(name="ps_z", bufs=2, space="PSUM"))

    # --- persistent activations (split per i-group for phase overlap) ---
    qT_g = [persist.tile([P, EC, NI], F16, name=f"qT{g}") for g in range(NG)]
    kT_g = [persist.tile([P, EC, NI], F16, name=f"kT{g}") for g in range(NG)]
    zT_sb = persist.tile([P, EC, S], F32R)  # normalized z^T
    # v natural layout + 64 ones columns (rows 64..127 of PV psum become l)
    v_g = [persist.tile([P, NSB // NG, HL, 2 * DH], F16, name=f"v{g}")
           for g in range(NG)]

    xq_r = xq.rearrange("(c p) s -> p c s", p=P)
    xk_r = xk.rearrange("(c p) s -> p c s", p=P)
    xv_r = xv.rearrange("(c p) s -> p c s", p=P)

    # --- first-half x loads emitted first so PE starts ASAP; weights and
    # constants loaded just-in-time on the same queue ---
    wk_sb = persist.tile([P, DC, E], F16)
    wq_sb = persist.tile([P, DC, E], F16)
    wv_sb = persist.tile([P, DC, E], F16)
    wo_sb = persist.tile([P, EC, D], F32R)
    bq_sb = persist.tile([P, EC], F32)
    bk_sb = persist.tile([P, EC], F32)
    bv_bc = persist.tile([P, E], F32)
    masks_sb = persist.tile([P, 4, 512], F16)

    def emit_kq(g):
        if g == 0:
            nc.sync.dma_start(out=wk_sb,
                              in_=wk.rearrange("(c p) e -> p c e", p=P))
            nc.sync.dma_start(out=bk_sb,
                              in_=bk.rearrange("(c p) -> p c", p=P))
        for nl in range(NI // 512):  # local 512-col chunks
            n = g * (NI // 512) + nl
            for x_r, w_sb, b_sb, scale, dstT in (
                (xk_r, wk_sb, bk_sb, 1.0, kT_g[g]),
                (xq_r, wq_sb, bq_sb, 1.0 / ATTN_SCALE, qT_g[g]),
            ):
                xs = xstage.tile([P, DC, 512], F16, tag="xs")
                nc.sync.dma_start(out=xs, in_=x_r[:, :, n * 512:(n + 1) * 512])
                if g == 0 and nl == 0 and dstT is kT_g[0]:
                    # interleave the q-weight loads behind the first k chunk
                    nc.sync.dma_start(
                        out=wq_sb, in_=wq.rearrange("(c p) e -> p c e", p=P))
                    nc.sync.dma_start(
                        out=bq_sb, in_=bq.rearrange("(c p) -> p c", p=P))
                for m in range(EC):
                    ps = ps_mm.tile([P, 512], F32, tag="mm")
                    for dc in range(DC):
                        nc.tensor.matmul(
                            ps,
                            lhsT=w_sb[:, dc, m * P:(m + 1) * P],
                            rhs=xs[:, dc, :],
                            start=(dc == 0),
                            stop=(dc == DC - 1),
                        )
                    # dstT = ps * scale + bias  (bias per-partition scalar)
                    nc.scalar.activation(
                        out=dstT[:, m, nl * 512:(nl + 1) * 512],
                        in_=ps,
                        func=AF.Identity,
                        bias=b_sb[:, m:m + 1],
                        scale=scale,
                    )

    def emit_v(g):
        if g == 0:
            nc.sync.dma_start(out=wv_sb,
                              in_=wv.rearrange("(c p) e -> p c e", p=P))
            bv_bcast_ap = bass.AP(tensor=bv.tensor, offset=bv.offset,
                                  ap=[[0, P]] + list(bv.ap))
            nc.sync.dma_start(out=bv_bc, in_=bv_bcast_ap)
        nsb_half = NSB // NG
        for sbl in range(nsb_half):
            sb = g * nsb_half + sbl
            xs = xvstage.tile([P, DC, P], F16, tag="xv")
            nc.sync.dma_start(out=xs, in_=xv_r[:, :, sb * P:(sb + 1) * P])
            ps = ps_mm.tile([P, E], F32, tag="mm")
            for dc in range(DC):
                nc.tensor.matmul(
                    ps,
                    lhsT=xs[:, dc, :],
                    rhs=wv_sb[:, dc, :],
                    start=(dc == 0),
                    stop=(dc == DC - 1),
                )
            nc.vector.tensor_add(
                out=v_g[g][:, sbl, :, 0:DH],
                in0=ps.rearrange("p (h e) -> p h e", h=HL),
                in1=bv_bc.rearrange("p (h e) -> p h e", h=HL),
            )
            # ones columns: psum * 0 + 1 (a memset would be illegal on f32r)
            nc.vector.tensor_scalar(
                out=v_g[g][:, sbl, :, DH:2 * DH],
                in0=ps.rearrange("p (h e) -> p h e", h=HL),
                scalar1=0.0,
                scalar2=1.0,
                op0=mybir.AluOpType.mult,
                op1=mybir.AluOpType.add,
            )
        if g == 0:
            nc.sync.dma_start(out=masks_sb, in_=masks)
            nc.sync.dma_start(out=wo_sb,
                              in_=wo.rearrange("(c p) d -> p c d", p=P))

    def emit_attn(g):
        jmax = (NI // P) * g + (NI // P)  # j-blocks 0..jmax-1 (8 or 16)
        for h in range(HL):
            hc, hb = h // 2, h % 2
            e0 = hb * DH  # partition base of this head's 64 dims
            # contributing j-blocks per 512-wide i-chunk (causal skip)
            # first 512-chunk each strip touches (fully-masked chunks skipped)
            def _ct(jb):
                t = jb - (NI // P) * g
                return 0 if t < 4 else 1

            contrib = [[jb for jb in range(jmax) if _ct(jb) <= c]
                       for c in range(2)]
            zps = [ps_z.tile([2 * DH, 512], F32, tag="z", name=f"zps{c}")
                   for c in range(2)]
            for jb in range(jmax):
                t = jb - (NI // P) * g  # >=0 on diagonal strips
                ct = _ct(jb)
                sps = ps_s.tile([P, NI], F32, tag="s")
                pt = ptpool.tile([P, NI], F16, tag="pt")
                zlo = max(0, t) * P
                for c in range(ct, 2):
                    c0 = c * 512
                    lo = max(zlo, c0)  # fp16: 1 cyc/row at any width, so
                    nc.tensor.matmul(  # trim to the 128-aligned boundary
                        sps[:, lo:c0 + 512],
                        lhsT=kT_g[jb // (NI // P)][
                            e0:e0 + DH, hc,
                            (jb % (NI // P)) * P:(jb % (NI // P) + 1) * P],
                        rhs=qT_g[g][e0:e0 + DH, hc, lo:c0 + 512],
                        start=True,
                        stop=True,
                    )
                nc.scalar.activation(out=pt[:, zlo:NI],
                                     in_=sps[:, zlo:NI], func=AF.Exp)
                if t >= 0:
                    # triangle mask on the diagonal 128 columns (GpSimd:
                    # SBUF-only op, keeps DVE free for norm/copies)
                    nc.gpsimd.tensor_mul(
                        out=pt[:, zlo:zlo + P],
                        in0=pt[:, zlo:zlo + P],
                        in1=masks_sb[:, 0, 0:P],
                    )
                for c in range(ct, 2):
                    c0 = c * 512
                    lo = max(zlo, c0)  # masked cols are simply never read
                    nc.tensor.matmul(
                        zps[c][:, lo - c0:512],
                        lhsT=v_g[jb // (NSB // NG)][
                            :, jb % (NSB // NG), h, :],
                        rhs=pt[:, lo:c0 + 512],
                        start=(jb == contrib[c][0]),
                        stop=(jb == contrib[c][-1]),
                    )
            # normalize: zT = z * (1/l); rows DH..2DH of zps all hold l
            for c in range(2):
                bcr = small.tile([DH, 512], F32, tag="bcr")
                nc.vector.reciprocal(bcr, zps[c][DH:2 * DH, :])
                icol = g * NI + c * 512
                nc.vector.tensor_mul(
                    out=zT_sb[e0:e0 + DH, hc, icol:icol + 512],
                    in0=zps[c][0:DH, :],
                    in1=bcr,
                )
        # output projection for the i-blocks of this group
        for ib in range((NI // P) * g, (NI // P) * g + NI // P):
            osb = outpool.tile([P, D], F32, tag="o")
            for d2 in range(2):
                ops = ps_mm.tile([P, 512], F32, tag="mm")
                for ec in range(EC):
                    nc.tensor.matmul(
                        ops,
                        lhsT=zT_sb[:, ec, ib * P:(ib + 1) * P],
                        rhs=wo_sb[:, ec, d2 * 512:(d2 + 1) * 512],
                        start=(ec == 0),
                        stop=(ec == EC - 1),
                    )
                if d2 == 0:
                    nc.vector.tensor_copy(
                        out=osb[:, d2 * 512:(d2 + 1) * 512], in_=ops)
                else:
                    nc.scalar.activation(
                        out=osb[:, d2 * 512:(d2 + 1) * 512], in_=ops,
                        func=AF.Copy)
            eng = nc.gpsimd if ib % 2 == 0 else nc.sync
            eng.dma_start(out=out[ib * P:(ib + 1) * P, :], in_=osb)

    emit_kq(0)
    emit_v(0)
    emit_kq(1)
    emit_v(1)
    emit_attn(0)
    emit_attn(1)


def build_nc():
    from contextlib import ExitStack

    nc = bass.Bass()
    xq = nc.dram_tensor("xq", [D, S], F16, kind="ExternalInput")[:]
    xk = nc.dram_tensor("xk", [D, S], F16, kind="ExternalInput")[:]
    xv = nc.dram_tensor("xv", [D, S], F16, kind="ExternalInput")[:]
    wq = nc.dram_tensor("wq", [D, E], F16, kind="ExternalInput")[:]
    wk = nc.dram_tensor("wk", [D, E], F16, kind="ExternalInput")[:]
    wv = nc.dram_tensor("wv", [D, E], F16, kind="ExternalInput")[:]
    wo = nc.dram_tensor("wo", [E, D], F32R, kind="ExternalInput")[:]
    bq = nc.dram_tensor("bq", [E], F32, kind="ExternalInput")[:]
    bk = nc.dram_tensor("bk", [E], F32, kind="ExternalInput")[:]
    bv = nc.dram_tensor("bv", [E], F32, kind="ExternalInput")[:]
    masks = nc.dram_tensor("masks", [P, 4, 512], F16, kind="ExternalInput")[:]
    out = nc.dram_tensor("out", [S, D], F32, kind="ExternalOutput")[:]
    with tile.TileContext(nc) as tc:
        with ExitStack() as ctx:
            _emit(ctx, tc, xq, xk, xv, wq, wk, wv, wo, bq, bk, bv, masks, out)
    return nc


_CACHE = {}


def _get_nc():
    if "nc" not in _CACHE:
        _CACHE["nc"] = build_nc()
    return _CACHE["nc"]


def make_in_maps(query_input, key_input, value_input, W_Q, W_K, W_V, W_O,
                 b_Q, b_K, b_V, b_O):
    qi = np.asarray(query_input, dtype=np.float32)
    ki = np.asarray(key_input, dtype=np.float32)
    vi = np.asarray(value_input, dtype=np.float32)
    W_Q = np.asarray(W_Q, dtype=np.float32)
    W_K = np.asarray(W_K, dtype=np.float32)
    W_V = np.asarray(W_V, dtype=np.float32)
    W_O = np.asarray(W_O, dtype=np.float32)
    b_Q = np.asarray(b_Q, dtype=np.float32)
    b_K = np.asarray(b_K, dtype=np.float32)
    b_V = np.asarray(b_V, dtype=np.float32)

    tri128 = np.triu(np.ones((P, P), dtype=np.float16))  # tri[j, i] = i >= j
    masks = np.ones((P, 4, 512), dtype=np.float16)
    for m in range(4):
        masks[:, m, :128 * m] = 0.0
        masks[:, m, 128 * m:128 * m + 128] = tri128
    xT = {}
    for b in range(B):
        xT[("q", b)] = np.ascontiguousarray(qi[b].T).astype(np.float16)
        xT[("k", b)] = np.ascontiguousarray(ki[b].T).astype(np.float16)
        xT[("v", b)] = np.ascontiguousarray(vi[b].T).astype(np.float16)

    in_maps = []
    for core in range(NCORES):
        b, hg = core // (NCORES // B), core % (NCORES // B)
        hs = slice(hg * HL, (hg + 1) * HL)
        in_maps.append({
            "xq": xT[("q", b)],
            "xk": xT[("k", b)],
            "xv": xT[("v", b)],
            "wq": np.ascontiguousarray(
                np.transpose(W_Q[hs], (1, 0, 2)).reshape(D, E)).astype(np.float16),
            "wk": np.ascontiguousarray(
                np.transpose(W_K[hs], (1, 0, 2)).reshape(D, E)).astype(np.float16),
            "wv": np.ascontiguousarray(
                np.transpose(W_V[hs], (1, 0, 2)).reshape(D, E)).astype(np.float16),
            "wo": _round_f32r(W_O[hs].reshape(E, D)),
            "bq": np.ascontiguousarray(
                (b_Q[hs].reshape(E) / ATTN_SCALE).astype(np.float32)),
            "bk": np.ascontiguousarray(b_K[hs].reshape(E)),
            "bv": np.ascontiguousarray(b_V[hs].reshape(E)),
            "masks": masks,
        })
    return in_maps


def gather_out(results, b_O):
    out = np.zeros((B, S, D), dtype=np.float64)
    for core in range(NCORES):
        out[core // (NCORES // B)] += results[core]["out"].astype(np.float64)
    out += np.asarray(b_O, dtype=np.float64)
    return out.astype(np.float32)


def kernel(query_input, key_input, value_input, W_Q, W_K, W_V, W_O,
           b_Q, b_K, b_V, b_O):
    nc = _get_nc()
    in_maps = make_in_maps(query_input, key_input, value_input,
                           W_Q, W_K, W_V, W_O, b_Q, b_K, b_V, b_O)
    res = run_bass_kernel_spmd(nc, in_maps, list(range(NCORES)))
    return gather_out(res.results, b_O)


def kernel_timed(inputs, trace_cores=None, **kwargs):
    """Like kernel() but traces and returns (out, BassKernelResults)."""
    nc = _get_nc()
    in_maps = make_in_maps(**inputs)
    res = run_bass_kernel_spmd(
        nc, in_maps, list(range(NCORES)), trace=True,
        trace_cores=trace_cores, **kwargs)
    return gather_out(res.results, inputs["b_O"]), res



# revision 15
# speedup vs baseline: 1.0715x; 1.0715x over previous
"""Trainium2 Bass kernel for multi-head causal attention.

Problem: B=2, S=2048, D=1024, H=16, DH=64 (fp32), causal attention with
QKV projections and output projection summed over heads.

Sharding: 8 cores = (batch b in {0,1}) x (head-group hg in {0..3}, 4 heads
each).  Each core computes a partial output sum over its 4 heads for its
batch; the host sums the 4 partials per batch and adds b_O.

v2 (fp8 DoubleRow): the Q/K projections and the score matmuls run in
fp8-e4m3 DoubleRow mode (0.5 cycles/row vs 1.0 for fp16), halving their
PE cost.  W_Q/W_K are pre-scaled by 64 on the host (keeps the fp8 values
out of the subnormal range), x_q/x_k are shipped fp8 in d-pair layout
[64p, 2, ...]; the projection PSUM result is written to SBUF as fp8
directly by DVE (scale 2^-5 + bias), then a cheap SBUF->SBUF DMA folds
the e-partition dim into (32 partitions, 2 pair columns) per head for the
DoubleRow score matmuls.  The leftover scale (64*2^-5)^2 = 4 is folded
into the Exp activation's scale together with 1/ATTN_SCALE (1/32 total).

V projection, PV, and the output projection stay fp16/f32r: emulation
shows fp8 there pushes rel err past the 2e-2 gate, while this config
measures ~1.1e-2 end to end.

The softmax denominator comes from 64 ones-columns appended to v (PSUM
partitions 64..127 of the PV matmul all hold l); normalization is a
single DVE divide.  Output partials are stored fp16 (halves the output
DMA) and summed on the host in float64.

A BIR post-processing patch (installed on import) hoists excess sync waits
off instructions into standalone EventSemaphore ops — walrus codegen allows
only 1 wait on the fused 4-byte-weight-load matmul encoding and few on
other opcodes, and Tile emits more.
"""

import sys

import numpy as np

for _p in ("/opt/trn_rl_repo",):
    if _p not in sys.path:
        sys.path.insert(0, _p)

import ml_dtypes

import concourse.bass as bass
import concourse.tile as tile
from concourse import mybir
from concourse.bass_utils import run_bass_kernel_spmd


def _hoist_matmul_waits(bir_json: bytes) -> bytes:
    """Move extra sync waits off instructions into EventSemaphore ops."""
    import orjson

    m = orjson.loads(bir_json)
    changed = False
    for fn in m.get("functions", []):
        for bb in fn.get("blocks", []):
            insts = bb.get("instructions", [])
            out = []
            for inst in insts:
                si = inst.get("sync_info") or {}
                waits = si.get("on_wait") or []
                if len(waits) > 1:
                    keep = waits[-1]
                    for wi, w in enumerate(waits[:-1]):
                        out.append({
                            "debug": inst.get("debug", 0),
                            "engine": inst["engine"],
                            "ins": [],
                            "name": f"{inst['name']}-hw{wi}",
                            "opcode": "EventSemaphore",
                            "outs": [],
                            "sync_info": {"on_update": [],
                                          "on_wait": [w]},
                        })
                    si["on_wait"] = [keep]
                    inst["sync_info"] = si
                    changed = True
                out.append(inst)
            bb["instructions"] = out
    if not changed:
        return bir_json
    return orjson.dumps(m)


def _install_bir_patch():
    from concourse import bass2jax as _b2j
    from concourse import bass_utils as _bu

    if getattr(_b2j, "_mm_wait_patch", False):
        return

    _orig = _bu.compile_bir_kernel

    def _patched(bir_json, tmpdir, neff_name="file.neff"):
        return _orig(_hoist_matmul_waits(bir_json), tmpdir, neff_name)

    _b2j.compile_bir_kernel = _patched
    _bu.compile_bir_kernel = _patched
    _b2j._mm_wait_patch = True


_install_bir_patch()

# Problem dims (hardcoded per harness contract).
B, S, D, H, DH = 2, 2048, 1024, 16, 64
ATTN_SCALE = 8.0
NCORES = 8
HL = H // (NCORES // B)  # 4 local heads per core
E = HL * DH              # 256 local head dims
P = 128
DC = D // P              # 8 contraction chunks
EC = E // P              # 2 e-chunks
NSB = S // P             # 16 s-blocks of 128
NI = 1024                # i-group width for score strips
NG = S // NI             # 2 i-groups
WSC = 64.0               # host-side W_Q/W_K scale (power of 2)
QSC = 2.0 ** -5          # q/k fp8 quantization scale
# exp argument scale: (WSC*QSC)^2 * ATTN_SCALE undone at the exp
ESC = 1.0 / ((WSC * QSC) ** 2 * ATTN_SCALE)
F32 = mybir.dt.float32
F32R = mybir.dt.float32r
F16 = mybir.dt.float16
F8 = mybir.dt.float8e4
AF = mybir.ActivationFunctionType
DR = mybir.MatmulPerfMode.DoubleRow
NP8 = ml_dtypes.float8_e4m3


def _round_f32r(arr):
    """Round an fp32 array to float32r (tfloat32) representable values."""
    from neuronxcc.starfish.support import dtype as nxd
    a = np.ascontiguousarray(np.asarray(arr, dtype=np.float32))
    return np.asarray(nxd.static_cast(a, dtype=nxd.float32r)).view(np.float32)


def _emit(ctx, tc, xq8, xk8, xv, wq8, wk8, wv, wo, bq, bk, bv, tri, out):
    nc = tc.nc

    persist = ctx.enter_context(tc.tile_pool(name="persist", bufs=1))
    xstage = ctx.enter_context(tc.tile_pool(name="xstage", bufs=4))
    xvstage = ctx.enter_context(tc.tile_pool(name="xvstage", bufs=3))
    ptpool = ctx.enter_context(tc.tile_pool(name="ptp", bufs=8))
    outpool = ctx.enter_context(tc.tile_pool(name="outp", bufs=4))
    small = ctx.enter_context(tc.tile_pool(name="small", bufs=6))
    # PSUM budget (8 banks of [128, 2KB]):
    #   ps_s: score strips [128, 1024] = 2 banks x 2 bufs = 4
    #   ps_mm: proj / outproj [128, <=512] = 1 bank x 2 bufs = 2
    #   ps_z: PV accumulators [128, 512] = 1 bank x 2 bufs = 2
    ps_s = ctx.enter_context(tc.tile_pool(name="ps_s", bufs=2, space="PSUM"))
    ps_mm = ctx.enter_context(tc.tile_pool(name="ps_mm", bufs=2, space="PSUM"))
    ps_z = ctx.enter_context(tc.tile_pool(name="ps_z", bufs=2, space="PSUM"))

    # --- persistent activations ---
    # q/k in fp8, raw projection layout [e-part, ec, s]
    q8raw = persist.tile([P, EC, S], F8)
    k8raw = persist.tile([P, EC, S], F8)
    # pair-folded for DoubleRow scores: per e-chunk hc a [64, 2, S] tile,
    # head 2hc+j at partitions j*32..j*32+32 (AP base must be in {0,32,64})
    q8t = [persist.tile([64, 2, S], F8, name=f"q8t{hc}") for hc in range(EC)]
    k8t = [persist.tile([64, 2, S], F8, name=f"k8t{hc}") for hc in range(EC)]
    zT_sb = persist.tile([P, EC, S], F32R)  # normalized z^T
    # v natural layout + 64 ones columns (rows 64..127 of PV psum become l)
    v_g = [persist.tile([P, NSB // NG, HL, 2 * DH], F16, name=f"v{g}")
           for g in range(NG)]

    wk_sb = persist.tile([64, DC, 2, E], F8)
    wq_sb = persist.tile([64, DC, 2, E], F8)
    wv_sb = persist.tile([P, DC, E], F16)
    wo_sb = persist.tile([P, EC, D], F32R)
    bq_sb = persist.tile([P, EC], F32)
    bk_sb = persist.tile([P, EC], F32)
    bv_bc = persist.tile([P, E], F32)
    tri_sb = persist.tile([P, P], F16)

    def emit_kq(g):
        if g == 0:
            nc.sync.dma_start(out=wk_sb, in_=wk8)
            nc.sync.dma_start(out=bk_sb,
                              in_=bk.rearrange("(c p) -> p c", p=P))
        for nl in range(NI // 512):  # local 512-col chunks
            n = g * (NI // 512) + nl
            for x8_r, w_sb, b_sb, raw in (
                (xk8, wk_sb, bk_sb, k8raw),
                (xq8, wq_sb, bq_sb, q8raw),
            ):
                xs = xstage.tile([64, DC, 2, 512], F8, tag="xs")
                nc.sync.dma_start(out=xs, in_=x8_r[n])
                if g == 0 and nl == 0 and raw is k8raw:
                    nc.sync.dma_start(out=wq_sb, in_=wq8)
                    nc.sync.dma_start(
                        out=bq_sb, in_=bq.rearrange("(c p) -> p c", p=P))
                for m in range(EC):
                    ps = ps_mm.tile([P, 512], F32, tag="mm")
                    for c in range(DC):
                        nc.tensor.matmul(
                            ps,
                            lhsT=w_sb[:, c, :, m * P:(m + 1) * P],
                            rhs=xs[:, c, :, :],
                            start=(c == 0),
                            stop=(c == DC - 1),
                            perf_mode=DR,
                        )
                    # raw = (ps + bias) * 2^-5, quantized to fp8
                    nc.vector.tensor_scalar(
                        out=raw[:, m, n * 512:(n + 1) * 512],
                        in0=ps,
                        scalar1=b_sb[:, m:m + 1],
                        scalar2=QSC,
                        op0=mybir.AluOpType.add,
                        op1=mybir.AluOpType.mult,
                    )
        # fold e-partitions into (32p, 2) pairs per head for DoubleRow:
        # partition p (0..127) of chunk hc -> head 2hc+p//64, out partition
        # p//2, pair col p%2 (heads 2hc, 2hc+1 at bases 0, 32 of tile hc).
        for raw, dst in ((k8raw, k8t), (q8raw, q8t)):
            for hc in range(EC):
                nc.sync.dma_start(
                    out=dst[hc][:, :, g * NI:(g + 1) * NI],
                    in_=raw[:, hc, g * NI:(g + 1) * NI],
                )

    def emit_v(g):
        if g == 0:
            nc.sync.dma_start(out=wv_sb, in_=wv)
            bv_bcast_ap = bass.AP(tensor=bv.tensor, offset=bv.offset,
                                  ap=[[0, P]] + list(bv.ap))
            nc.sync.dma_start(out=bv_bc, in_=bv_bcast_ap)
        nsb_half = NSB // NG
        for nl in range(NI // 512):  # one xv chunk per 512 cols
            n = g * (NI // 512) + nl
            xs = xvstage.tile([P, DC, 512], F16, tag="xv")
            nc.sync.dma_start(out=xs, in_=xv[n])
            for sl in range(512 // P):
                sbl = nl * (512 // P) + sl
                ps = ps_mm.tile([P, E], F32, tag="mm")
                for dc in range(DC):
                    nc.tensor.matmul(
                        ps,
                        lhsT=xs[:, dc, sl * P:(sl + 1) * P],
                        rhs=wv_sb[:, dc, :],
                        start=(dc == 0),
                        stop=(dc == DC - 1),
                    )
                nc.vector.tensor_add(
                    out=v_g[g][:, sbl, :, 0:DH],
                    in0=ps.rearrange("p (h e) -> p h e", h=HL),
                    in1=bv_bc.rearrange("p (h e) -> p h e", h=HL),
                )
                # ones columns: psum * 0 + 1
                nc.vector.tensor_scalar(
                    out=v_g[g][:, sbl, :, DH:2 * DH],
                    in0=ps.rearrange("p (h e) -> p h e", h=HL),
                    scalar1=0.0,
                    scalar2=1.0,
                    op0=mybir.AluOpType.mult,
                    op1=mybir.AluOpType.add,
                )
        if g == 0:
            nc.sync.dma_start(out=tri_sb, in_=tri)
            nc.sync.dma_start(out=wo_sb,
                              in_=wo.rearrange("(c p) d -> p c d", p=P))

    def emit_attn(g):
        jmax = (NI // P) * g + (NI // P)  # j-blocks 0..jmax-1 (8 or 16)
        for h in range(HL):
            hc, hbit = h // 2, h % 2
            hb = hbit * 32  # partition base of this head in q8t[hc]/k8t[hc]
            e0 = hbit * DH  # partition base in zT (layout [e0+.., hc, s])

            def _ct(jb):
                t = jb - (NI // P) * g
                return 0 if t < 4 else 1

            contrib = [[jb for jb in range(jmax) if _ct(jb) <= c]
                       for c in range(2)]
            zps = [ps_z.tile([2 * DH, 512], F32, tag="z", name=f"zps{c}")
                   for c in range(2)]
            for jb in range(jmax):
                t = jb - (NI // P) * g  # >=0 on diagonal strips
                ct = _ct(jb)
                sps = ps_s.tile([P, NI], F32, tag="s")
                pt = ptpool.tile([P, NI], F16, tag="pt")
                zlo = max(0, t) * P
                for c in range(ct, 2):
                    c0 = c * 512
                    lo = max(zlo, c0)
                    nc.tensor.matmul(
                        sps[:, lo:c0 + 512],
                        lhsT=k8t[hc][hb:hb + 32, :, jb * P:(jb + 1) * P],
                        rhs=q8t[hc][hb:hb + 32, :,
                                    g * NI + lo:g * NI + c0 + 512],
                        start=True,
                        stop=True,
                        perf_mode=DR,
                    )
                nc.scalar.activation(out=pt[:, zlo:NI],
                                     in_=sps[:, zlo:NI], func=AF.Exp,
                                     scale=ESC)
                if t >= 0:
                    # triangle mask on the diagonal 128 columns (GpSimd:
                    # SBUF-only op, keeps DVE free)
                    nc.gpsimd.tensor_mul(
                        out=pt[:, zlo:zlo + P],
                        in0=pt[:, zlo:zlo + P],
                        in1=tri_sb,
                    )
                for c in range(ct, 2):
                    c0 = c * 512
                    lo = max(zlo, c0)  # masked cols are simply never read
                    nc.tensor.matmul(
                        zps[c][:, lo - c0:512],
                        lhsT=v_g[jb // (NSB // NG)][
                            :, jb % (NSB // NG), h, :],
                        rhs=pt[:, lo:c0 + 512],
                        start=(jb == contrib[c][0]),
                        stop=(jb == contrib[c][-1]),
                    )
            # normalize: zT = z * (1/l); rows DH..2DH of zps all hold l
            for c in range(2):
                bcr = small.tile([DH, 512], F32, tag="bcr")
                nc.vector.reciprocal(bcr, zps[c][DH:2 * DH, :])
                icol = g * NI + c * 512
                nc.vector.tensor_mul(
                    out=zT_sb[e0:e0 + DH, hc, icol:icol + 512],
                    in0=zps[c][0:DH, :],
                    in1=bcr,
                )
        # output projection for the i-blocks of this group
        for ib in range((NI // P) * g, (NI // P) * g + NI // P):
            osb = outpool.tile([P, D], F16, tag="o")
            for d2 in range(2):
                ops = ps_mm.tile([P, 512], F32, tag="mm")
                for ec in range(EC):
                    nc.tensor.matmul(
                        ops,
                        lhsT=zT_sb[:, ec, ib * P:(ib + 1) * P],
                        rhs=wo_sb[:, ec, d2 * 512:(d2 + 1) * 512],
                        start=(ec == 0),
                        stop=(ec == EC - 1),
                    )
                if d2 == 0:
                    nc.vector.tensor_copy(
                        out=osb[:, d2 * 512:(d2 + 1) * 512], in_=ops)
                else:
                    nc.scalar.activation(
                        out=osb[:, d2 * 512:(d2 + 1) * 512], in_=ops,
                        func=AF.Copy)
            eng = nc.gpsimd if ib % 2 == 0 else nc.sync
            eng.dma_start(out=out[ib * P:(ib + 1) * P, :], in_=osb)

    emit_kq(0)
    emit_v(0)
    emit_kq(1)
    emit_v(1)
    emit_attn(0)
    emit_attn(1)


def build_nc():
    from contextlib import ExitStack

    nc = bass.Bass()
    # x for q/k: fp8 in DoubleRow d-pair layout, chunked by 512 s-columns:
    # [n, p, c, b, s] with d = c*128 + b*64 + p (64 partitions, base 0)
    xq8 = nc.dram_tensor("xq8", [S // 512, 64, DC, 2, 512], F8,
                         kind="ExternalInput")[:]
    xk8 = nc.dram_tensor("xk8", [S // 512, 64, DC, 2, 512], F8,
                         kind="ExternalInput")[:]
    # x for v: fp16 [n, p, dc, s] with d = dc*128 + p
    xv = nc.dram_tensor("xv", [S // 512, P, DC, 512], F16,
                        kind="ExternalInput")[:]
    # w for q/k: fp8 pre-scaled by WSC, same d-pair layout
    wq8 = nc.dram_tensor("wq8", [64, DC, 2, E], F8,
                         kind="ExternalInput")[:]
    wk8 = nc.dram_tensor("wk8", [64, DC, 2, E], F8,
                         kind="ExternalInput")[:]
    wv = nc.dram_tensor("wv", [P, DC, E], F16, kind="ExternalInput")[:]
    wo = nc.dram_tensor("wo", [E, D], F32R, kind="ExternalInput")[:]
    bq = nc.dram_tensor("bq", [E], F32, kind="ExternalInput")[:]
    bk = nc.dram_tensor("bk", [E], F32, kind="ExternalInput")[:]
    bv = nc.dram_tensor("bv", [E], F32, kind="ExternalInput")[:]
    tri = nc.dram_tensor("tri", [P, P], F16, kind="ExternalInput")[:]
    out = nc.dram_tensor("out", [S, D], F16, kind="ExternalOutput")[:]
    with tile.TileContext(nc) as tc:
        with ExitStack() as ctx:
            _emit(ctx, tc, xq8, xk8, xv, wq8, wk8, wv, wo, bq, bk, bv, tri,
                  out)
    return nc


_CACHE = {}


def _get_nc():
    if "nc" not in _CACHE:
        _CACHE["nc"] = build_nc()
    return _CACHE["nc"]


def make_in_maps(query_input, key_input, value_input, W_Q, W_K, W_V, W_O,
                 b_Q, b_K, b_V, b_O):
    qi = np.asarray(query_input, dtype=np.float32)
    ki = np.asarray(key_input, dtype=np.float32)
    vi = np.asarray(value_input, dtype=np.float32)
    W_Q = np.asarray(W_Q, dtype=np.float32)
    W_K = np.asarray(W_K, dtype=np.float32)
    W_V = np.asarray(W_V, dtype=np.float32)
    W_O = np.asarray(W_O, dtype=np.float32)
    b_Q = np.asarray(b_Q, dtype=np.float32)
    b_K = np.asarray(b_K, dtype=np.float32)
    b_V = np.asarray(b_V, dtype=np.float32)

    tri128 = np.triu(np.ones((P, P), dtype=np.float16))  # tri[j, i] = i >= j

    def pair_x2(xT8):
        # xT8: [D, S] fp8 -> [n, p, c, b, 512] with d = c*128 + b*64 + p
        a = xT8.reshape(DC, 2, 64, S // 512, 512)  # c b p n s
        a = a.transpose(3, 2, 0, 1, 4)             # n p c b s
        return np.ascontiguousarray(a)

    def pair_w(w8):
        # w8: [D, E] fp8 -> [p, c, b, E] with d = c*128 + b*64 + p
        a = w8.reshape(DC, 2, 64, E)  # c b p e
        a = a.transpose(2, 0, 1, 3)   # p c b e
        return np.ascontiguousarray(a)

    xT8 = {}
    xTv = {}
    for b in range(B):
        xT8[("q", b)] = pair_x2(
            np.ascontiguousarray(qi[b].T).astype(NP8))
        xT8[("k", b)] = pair_x2(
            np.ascontiguousarray(ki[b].T).astype(NP8))
        # v path: [n, p, dc, 512] fp16 with d = dc*128 + p
        a = np.ascontiguousarray(vi[b].T).astype(np.float16)
        a = a.reshape(DC, P, S // 512, 512).transpose(2, 1, 0, 3)
        xTv[b] = np.ascontiguousarray(a)

    in_maps = []
    for core in range(NCORES):
        b, hg = core // (NCORES // B), core % (NCORES // B)
        hs = slice(hg * HL, (hg + 1) * HL)
        wq_flat = np.transpose(W_Q[hs], (1, 0, 2)).reshape(D, E)
        wk_flat = np.transpose(W_K[hs], (1, 0, 2)).reshape(D, E)
        wv_flat = np.transpose(W_V[hs], (1, 0, 2)).reshape(D, E)
        in_maps.append({
            "xq8": xT8[("q", b)],
            "xk8": xT8[("k", b)],
            "xv": xTv[b],
            "wq8": pair_w((wq_flat * WSC).astype(NP8)),
            "wk8": pair_w((wk_flat * WSC).astype(NP8)),
            "wv": np.ascontiguousarray(
                wv_flat.reshape(DC, P, E).transpose(1, 0, 2)).astype(
                    np.float16),
            "wo": _round_f32r(W_O[hs].reshape(E, D)),
            "bq": np.ascontiguousarray(
                (b_Q[hs].reshape(E) * WSC).astype(np.float32)),
            "bk": np.ascontiguousarray(
                (b_K[hs].reshape(E) * WSC).astype(np.float32)),
            "bv": np.ascontiguousarray(b_V[hs].reshape(E)),
            "tri": tri128,
        })
    return in_maps


def gather_out(results, b_O):
    out = np.zeros((B, S, D), dtype=np.float64)
    for core in range(NCORES):
        out[core // (NCORES // B)] += results[core]["out"].astype(np.float64)
    out += np.asarray(b_O, dtype=np.float64)
    return out.astype(np.float32)


def kernel(query_input, key_input, value_input, W_Q, W_K, W_V, W_O,
           b_Q, b_K, b_V, b_O):
    nc = _get_nc()
    in_maps = make_in_maps(query_input, key_input, value_input,
                           W_Q, W_K, W_V, W_O, b_Q, b_K, b_V, b_O)
    res = run_bass_kernel_spmd(nc, in_maps, list(range(NCORES)))
    return gather_out(res.results, b_O)


def kernel_timed(inputs, trace_cores=None, **kwargs):
    """Like kernel() but traces and returns (out, BassKernelResults)."""
    nc = _get_nc()
    in_maps = make_in_maps(**inputs)
    res = run_bass_kernel_spmd(
        nc, in_maps, list(range(NCORES)), trace=True,
        trace_cores=trace_cores, **kwargs)
    return gather_out(res.results, inputs["b_O"]), res


# revision 23
# speedup vs baseline: 1.1360x; 1.0602x over previous
"""Trainium2 Bass kernel for multi-head causal attention.

Problem: B=2, S=2048, D=1024, H=16, DH=64 (fp32), causal attention with
QKV projections and output projection summed over heads.

Sharding: 8 cores = (batch b in {0,1}) x (head-group hg in {0..3}, 4 heads
each).  Each core computes a partial output sum over its 4 heads for its
batch; the host sums the 4 partials per batch and adds b_O.

v2 (fp8 DoubleRow): the Q/K projections and the score matmuls run in
fp8-e4m3 DoubleRow mode (0.5 cycles/row vs 1.0 for fp16), halving their
PE cost.  W_Q/W_K are pre-scaled by 64 on the host (keeps the fp8 values
out of the subnormal range), x_q/x_k are shipped fp8 in d-pair layout
[64p, 2, ...]; the projection PSUM result is written to SBUF as fp8
directly by DVE (scale 2^-5 + bias), then a cheap SBUF->SBUF DMA folds
the e-partition dim into (32 partitions, 2 pair columns) per head for the
DoubleRow score matmuls.  The leftover scale (64*2^-5)^2 = 4 is folded
into the Exp activation's scale together with 1/ATTN_SCALE (1/32 total).

V projection, PV, and the output projection stay fp16/f32r: emulation
shows fp8 there pushes rel err past the 2e-2 gate, while this config
measures ~1.1e-2 end to end.

The softmax denominator comes from 64 ones-columns appended to v (PSUM
partitions 64..127 of the PV matmul all hold l); normalization is a
single DVE divide.  Output partials are stored fp16 (halves the output
DMA) and summed on the host in float64.

A BIR post-processing patch (installed on import) hoists excess sync waits
off instructions into standalone EventSemaphore ops — walrus codegen allows
only 1 wait on the fused 4-byte-weight-load matmul encoding and few on
other opcodes, and Tile emits more.
"""

import sys

import numpy as np

for _p in ("/opt/trn_rl_repo",):
    if _p not in sys.path:
        sys.path.insert(0, _p)

import ml_dtypes

import concourse.bass as bass
import concourse.tile as tile
from concourse import mybir
from concourse.bass_utils import run_bass_kernel_spmd


def _hoist_matmul_waits(bir_json: bytes) -> bytes:
    """Move extra sync waits off instructions into EventSemaphore ops."""
    import orjson

    m = orjson.loads(bir_json)
    changed = False
    for fn in m.get("functions", []):
        for bb in fn.get("blocks", []):
            insts = bb.get("instructions", [])
            out = []
            for inst in insts:
                si = inst.get("sync_info") or {}
                waits = si.get("on_wait") or []
                if len(waits) > 1:
                    keep = waits[-1]
                    for wi, w in enumerate(waits[:-1]):
                        out.append({
                            "debug": inst.get("debug", 0),
                            "engine": inst["engine"],
                            "ins": [],
                            "name": f"{inst['name']}-hw{wi}",
                            "opcode": "EventSemaphore",
                            "outs": [],
                            "sync_info": {"on_update": [],
                                          "on_wait": [w]},
                        })
                    si["on_wait"] = [keep]
                    inst["sync_info"] = si
                    changed = True
                out.append(inst)
            bb["instructions"] = out
    if not changed:
        return bir_json
    return orjson.dumps(m)


def _install_bir_patch():
    from concourse import bass2jax as _b2j
    from concourse import bass_utils as _bu

    if getattr(_b2j, "_mm_wait_patch", False):
        return

    _orig = _bu.compile_bir_kernel

    def _patched(bir_json, tmpdir, neff_name="file.neff"):
        return _orig(_hoist_matmul_waits(bir_json), tmpdir, neff_name)

    _b2j.compile_bir_kernel = _patched
    _bu.compile_bir_kernel = _patched
    _b2j._mm_wait_patch = True


_install_bir_patch()

# Problem dims (hardcoded per harness contract).
B, S, D, H, DH = 2, 2048, 1024, 16, 64
ATTN_SCALE = 8.0
NCORES = 8
HL = H // (NCORES // B)  # 4 local heads per core
E = HL * DH              # 256 local head dims
P = 128
DC = D // P              # 8 contraction chunks
EC = E // P              # 2 e-chunks
NSB = S // P             # 16 s-blocks of 128
NI = 1024                # i-group width for score strips
NG = S // NI             # 2 i-groups
WSC = 64.0               # host-side W_Q/W_K scale (power of 2)
QSC = 2.0 ** -5          # q/k fp8 quantization scale
# exp argument scale: (WSC*QSC)^2 * ATTN_SCALE undone at the exp
ESC = 1.0 / ((WSC * QSC) ** 2 * ATTN_SCALE)
F32 = mybir.dt.float32
F32R = mybir.dt.float32r
F16 = mybir.dt.float16
F8 = mybir.dt.float8e4
AF = mybir.ActivationFunctionType
DR = mybir.MatmulPerfMode.DoubleRow
NP8 = ml_dtypes.float8_e4m3


def _round_f32r(arr):
    """Round an fp32 array to float32r (tfloat32) representable values."""
    from neuronxcc.starfish.support import dtype as nxd
    a = np.ascontiguousarray(np.asarray(arr, dtype=np.float32))
    return np.asarray(nxd.static_cast(a, dtype=nxd.float32r)).view(np.float32)


NPRE = 2  # g=1 j-blocks whose scores+exp are front-loaded


def _emit(ctx, tc, xq8, xk8, xv, wq8, wk8, wv, wo, bq, bk, bv, tri, out):
    nc = tc.nc

    persist = ctx.enter_context(tc.tile_pool(name="persist", bufs=1))
    xstage = ctx.enter_context(tc.tile_pool(name="xstage", bufs=2))
    xvstage = ctx.enter_context(tc.tile_pool(name="xvstage", bufs=2))
    ptpool = ctx.enter_context(tc.tile_pool(name="ptp", bufs=6))
    outpool = ctx.enter_context(tc.tile_pool(name="outp", bufs=4))
    small = ctx.enter_context(tc.tile_pool(name="small", bufs=2))
    # PSUM budget (8 banks of [128, 2KB]):
    #   ps_s: score strips [128, 1024] = 2 banks x 2 bufs = 4
    #   ps_mm: proj / outproj [128, <=512] = 1 bank x 2 bufs = 2
    #   ps_z: PV accumulators [128, 512] = 1 bank x 2 bufs = 2
    ps_s = ctx.enter_context(tc.tile_pool(name="ps_s", bufs=2, space="PSUM"))
    ps_mm = ctx.enter_context(tc.tile_pool(name="ps_mm", bufs=2, space="PSUM"))
    ps_z = ctx.enter_context(tc.tile_pool(name="ps_z", bufs=2, space="PSUM"))

    # --- persistent activations ---
    # q/k in fp8, raw projection layout [e-part, ec, s]
    q8raw = persist.tile([P, EC, S], F8)
    k8raw = persist.tile([P, EC, S], F8)
    # pair-folded for DoubleRow scores: per e-chunk hc a [64, 2, S] tile,
    # head 2hc+j at partitions j*32..j*32+32 (AP base must be in {0,32,64})
    q8t = [persist.tile([64, 2, S], F8, name=f"q8t{hc}") for hc in range(EC)]
    k8t = [persist.tile([64, 2, S], F8, name=f"k8t{hc}") for hc in range(EC)]
    zT_sb = persist.tile([P, EC, S], F16)  # normalized z^T
    # v natural layout + 64 ones columns (rows 64..127 of PV psum become l)
    v_g = [persist.tile([P, NSB // NG, HL, 2 * DH], F16, name=f"v{g}")
           for g in range(NG)]
    # front-loaded pattern tiles (g0 fully, g1 j-blocks < NPRE)
    pt_g0 = [[persist.tile([P, NI], F16, name=f"pt0_{h}_{jb}")
              for jb in range(NI // P)] for h in range(HL)]
    pt_g1e = [[persist.tile([P, NI], F16, name=f"pt1_{h}_{jb}")
               for jb in range(NPRE)] for h in range(HL)]

    wk_sb = persist.tile([64, DC, 2, E], F8)
    wq_sb = persist.tile([64, DC, 2, E], F8)
    wv_sb = persist.tile([P, DC, E], F16)
    wo_sb = persist.tile([P, EC, D], F16)
    bq_sb = persist.tile([P, EC], F32)
    bk_sb = persist.tile([P, EC], F32)
    bv_bc = persist.tile([P, E], F32)
    tri_sb = persist.tile([P, P], F16)

    def emit_kq(g):
        if g == 0:
            # weights on the gpsimd DMA queue so they land in parallel with
            # the first x chunk on SP
            nc.gpsimd.dma_start(out=wk_sb, in_=wk8)
            nc.gpsimd.dma_start(out=wq_sb, in_=wq8)
        for nl in range(NI // 512):  # local 512-col chunks
            n = g * (NI // 512) + nl
            for x8_r, w_sb, b_sb, raw, dst in (
                (xk8, wk_sb, bk_sb, k8raw, k8t),
                (xq8, wq_sb, bq_sb, q8raw, q8t),
            ):
                xs = xstage.tile([64, DC, 2, 512], F8, tag="xs")
                nc.sync.dma_start(out=xs, in_=x8_r[n])
                if g == 0 and nl == 0:
                    nc.sync.dma_start(
                        out=(bk_sb if raw is k8raw else bq_sb),
                        in_=(bk if raw is k8raw else bq).rearrange(
                            "(c p) -> p c", p=P))
                for m in range(EC):
                    ps = ps_mm.tile([P, 512], F32, tag="mm")
                    for c in range(DC):
                        nc.tensor.matmul(
                            ps,
                            lhsT=w_sb[:, c, :, m * P:(m + 1) * P],
                            rhs=xs[:, c, :, :],
                            start=(c == 0),
                            stop=(c == DC - 1),
                            perf_mode=DR,
                        )
                    # raw = (ps + bias) * 2^-5, quantized to fp8
                    nc.vector.tensor_scalar(
                        out=raw[:, m, n * 512:(n + 1) * 512],
                        in0=ps,
                        scalar1=b_sb[:, m:m + 1],
                        scalar2=QSC,
                        op0=mybir.AluOpType.add,
                        op1=mybir.AluOpType.mult,
                    )
                    # fold e-partitions into (32p, 2) pairs for DoubleRow:
                    # partition p -> out partition p//2, pair col p%2
                    # (heads 2m, 2m+1 at bases 0, 32 of tile m)
                    nc.sync.dma_start(
                        out=dst[m][:, :, n * 512:(n + 1) * 512],
                        in_=raw[:, m, n * 512:(n + 1) * 512],
                    )

    def emit_v(g):
        # ones columns for the whole group in one memset (fp16 is legal)
        nc.vector.memset(v_g[g][:, :, :, DH:2 * DH], 1.0)
        for nl in range(NI // 512):  # one xv chunk per 512 cols
            n = g * (NI // 512) + nl
            xs = xvstage.tile([P, DC, 512], F16, tag="xv")
            nc.gpsimd.dma_start(out=xs, in_=xv[n])
            for sl in range(512 // P):
                sbl = nl * (512 // P) + sl
                ps = ps_mm.tile([P, E], F32, tag="mm")
                for dc in range(DC):
                    nc.tensor.matmul(
                        ps,
                        lhsT=xs[:, dc, sl * P:(sl + 1) * P],
                        rhs=wv_sb[:, dc, :],
                        start=(dc == 0),
                        stop=(dc == DC - 1),
                    )
                nc.vector.tensor_add(
                    out=v_g[g][:, sbl, :, 0:DH],
                    in0=ps.rearrange("p (h e) -> p h e", h=HL),
                    in1=bv_bc.rearrange("p (h e) -> p h e", h=HL),
                )

    def emit_scores(g, jbs, pt_dst):
        """Scores + exp (+ causal mask) for strips of group g into pt_dst."""
        for h in range(HL):
            hc, hbit = h // 2, h % 2
            hb = hbit * 32
            for jb in jbs:
                t = jb - (NI // P) * g
                ct = 0 if t < 4 else 1
                sps = ps_s.tile([P, NI], F32, tag="s")
                pt = pt_dst(h, jb)
                zlo = max(0, t) * P
                for c in range(ct, 2):
                    c0 = c * 512
                    lo = max(zlo, c0)
                    nc.tensor.matmul(
                        sps[:, lo:c0 + 512],
                        lhsT=k8t[hc][hb:hb + 32, :, jb * P:(jb + 1) * P],
                        rhs=q8t[hc][hb:hb + 32, :,
                                    g * NI + lo:g * NI + c0 + 512],
                        start=True,
                        stop=True,
                        perf_mode=DR,
                    )
                nc.scalar.activation(out=pt[:, zlo:NI],
                                     in_=sps[:, zlo:NI], func=AF.Exp,
                                     scale=ESC)
                if t >= 0:
                    # triangle mask on the diagonal 128 columns (GpSimd:
                    # SBUF-only op, keeps DVE free)
                    nc.gpsimd.tensor_mul(
                        out=pt[:, zlo:zlo + P],
                        in0=pt[:, zlo:zlo + P],
                        in1=tri_sb,
                    )

    def emit_pv_norm(g, pt_src):
        """PV + normalization for all heads of group g.

        pt_src(h, jb) returns the pattern tile, or None if its scores+exp
        still need to be emitted here (tail of g=1).
        """
        jmax = (NI // P) * g + (NI // P)
        for h in range(HL):
            hc, hbit = h // 2, h % 2
            hb = hbit * 32
            e0 = hbit * DH

            def _ct(jb):
                t = jb - (NI // P) * g
                return 0 if t < 4 else 1

            contrib = [[jb for jb in range(jmax) if _ct(jb) <= c]
                       for c in range(2)]
            zps = [ps_z.tile([2 * DH, 512], F32, tag="z", name=f"zps{c}")
                   for c in range(2)]
            for jb in range(jmax):
                t = jb - (NI // P) * g
                ct = _ct(jb)
                zlo = max(0, t) * P
                pt = pt_src(h, jb)
                if pt is None:
                    sps = ps_s.tile([P, NI], F32, tag="s")
                    pt = ptpool.tile([P, NI], F16, tag="pt")
                    for c in range(ct, 2):
                        c0 = c * 512
                        lo = max(zlo, c0)
                        nc.tensor.matmul(
                            sps[:, lo:c0 + 512],
                            lhsT=k8t[hc][hb:hb + 32, :,
                                         jb * P:(jb + 1) * P],
                            rhs=q8t[hc][hb:hb + 32, :,
                                        g * NI + lo:g * NI + c0 + 512],
                            start=True,
                            stop=True,
                            perf_mode=DR,
                        )
                    nc.scalar.activation(out=pt[:, zlo:NI],
                                         in_=sps[:, zlo:NI], func=AF.Exp,
                                         scale=ESC)
                    if t >= 0:
                        nc.gpsimd.tensor_mul(
                            out=pt[:, zlo:zlo + P],
                            in0=pt[:, zlo:zlo + P],
                            in1=tri_sb,
                        )
                for c in range(ct, 2):
                    c0 = c * 512
                    lo = max(zlo, c0)  # masked cols are simply never read
                    nc.tensor.matmul(
                        zps[c][:, lo - c0:512],
                        lhsT=v_g[jb // (NSB // NG)][
                            :, jb % (NSB // NG), h, :],
                        rhs=pt[:, lo:c0 + 512],
                        start=(jb == contrib[c][0]),
                        stop=(jb == contrib[c][-1]),
                    )
            # normalize: zT = z * (1/l); rows DH..2DH of zps all hold l
            for c in range(2):
                bcr = small.tile([DH, 512], F32, tag="bcr")
                nc.vector.reciprocal(bcr, zps[c][DH:2 * DH, :])
                icol = g * NI + c * 512
                nc.vector.tensor_mul(
                    out=zT_sb[e0:e0 + DH, hc, icol:icol + 512],
                    in0=zps[c][0:DH, :],
                    in1=bcr,
                )

    def emit_outproj(g):
        for ib in range((NI // P) * g, (NI // P) * g + NI // P):
            osb = outpool.tile([P, D], F16, tag="o")
            for d2 in range(2):
                ops = ps_mm.tile([P, 512], F32, tag="mm")
                for ec in range(EC):
                    nc.tensor.matmul(
                        ops,
                        lhsT=zT_sb[:, ec, ib * P:(ib + 1) * P],
                        rhs=wo_sb[:, ec, d2 * 512:(d2 + 1) * 512],
                        start=(ec == 0),
                        stop=(ec == EC - 1),
                    )
                if d2 == 0:
                    nc.vector.tensor_copy(
                        out=osb[:, d2 * 512:(d2 + 1) * 512], in_=ops)
                else:
                    nc.scalar.activation(
                        out=osb[:, d2 * 512:(d2 + 1) * 512], in_=ops,
                        func=AF.Copy)
            eng = nc.gpsimd if ib % 2 == 0 else nc.sync
            eng.dma_start(out=out[ib * P:(ib + 1) * P, :], in_=osb)

    # secondary loads early on the gpsimd queue (wk/wq go first inside
    # emit_kq(0) below)
    def emit_v_loads(g):
        if g == 0:
            nc.gpsimd.dma_start(out=wv_sb, in_=wv)
            nc.gpsimd.dma_start(out=tri_sb, in_=tri)
            bv_bcast_ap = bass.AP(tensor=bv.tensor, offset=bv.offset,
                                  ap=[[0, P]] + list(bv.ap))
            nc.gpsimd.dma_start(out=bv_bc, in_=bv_bcast_ap)
        else:
            nc.gpsimd.dma_start(out=wo_sb, in_=wo)

    emit_kq(0)
    emit_v_loads(0)
    emit_scores(0, range(NI // P), lambda h, jb: pt_g0[h][jb])
    emit_kq(1)
    emit_v_loads(1)
    emit_scores(1, range(NPRE), lambda h, jb: pt_g1e[h][jb])
    emit_v(0)
    emit_pv_norm(0, lambda h, jb: pt_g0[h][jb])
    emit_v(1)
    emit_outproj(0)
    emit_pv_norm(1, lambda h, jb: pt_g1e[h][jb] if jb < NPRE else None)
    emit_outproj(1)


def build_nc():
    from contextlib import ExitStack

    nc = bass.Bass()
    # x for q/k: fp8 in DoubleRow d-pair layout, chunked by 512 s-columns:
    # [n, p, c, b, s] with d = c*128 + b*64 + p (64 partitions, base 0)
    xq8 = nc.dram_tensor("xq8", [S // 512, 64, DC, 2, 512], F8,
                         kind="ExternalInput")[:]
    xk8 = nc.dram_tensor("xk8", [S // 512, 64, DC, 2, 512], F8,
                         kind="ExternalInput")[:]
    # x for v: fp16 [n, p, dc, s] with d = dc*128 + p
    xv = nc.dram_tensor("xv", [S // 512, P, DC, 512], F16,
                        kind="ExternalInput")[:]
    # w for q/k: fp8 pre-scaled by WSC, same d-pair layout
    wq8 = nc.dram_tensor("wq8", [64, DC, 2, E], F8,
                         kind="ExternalInput")[:]
    wk8 = nc.dram_tensor("wk8", [64, DC, 2, E], F8,
                         kind="ExternalInput")[:]
    wv = nc.dram_tensor("wv", [P, DC, E], F16, kind="ExternalInput")[:]
    wo = nc.dram_tensor("wo", [P, EC, D], F16, kind="ExternalInput")[:]
    bq = nc.dram_tensor("bq", [E], F32, kind="ExternalInput")[:]
    bk = nc.dram_tensor("bk", [E], F32, kind="ExternalInput")[:]
    bv = nc.dram_tensor("bv", [E], F32, kind="ExternalInput")[:]
    tri = nc.dram_tensor("tri", [P, P], F16, kind="ExternalInput")[:]
    out = nc.dram_tensor("out", [S, D], F16, kind="ExternalOutput")[:]
    with tile.TileContext(nc) as tc:
        with ExitStack() as ctx:
            _emit(ctx, tc, xq8, xk8, xv, wq8, wk8, wv, wo, bq, bk, bv, tri,
                  out)
    return nc


_CACHE = {}


def _get_nc():
    if "nc" not in _CACHE:
        _CACHE["nc"] = build_nc()
    return _CACHE["nc"]


def make_in_maps(query_input, key_input, value_input, W_Q, W_K, W_V, W_O,
                 b_Q, b_K, b_V, b_O):
    qi = np.asarray(query_input, dtype=np.float32)
    ki = np.asarray(key_input, dtype=np.float32)
    vi = np.asarray(value_input, dtype=np.float32)
    W_Q = np.asarray(W_Q, dtype=np.float32)
    W_K = np.asarray(W_K, dtype=np.float32)
    W_V = np.asarray(W_V, dtype=np.float32)
    W_O = np.asarray(W_O, dtype=np.float32)
    b_Q = np.asarray(b_Q, dtype=np.float32)
    b_K = np.asarray(b_K, dtype=np.float32)
    b_V = np.asarray(b_V, dtype=np.float32)

    tri128 = np.triu(np.ones((P, P), dtype=np.float16))  # tri[j, i] = i >= j

    def pair_x2(xT8):
        # xT8: [D, S] fp8 -> [n, p, c, b, 512] with d = c*128 + b*64 + p
        a = xT8.reshape(DC, 2, 64, S // 512, 512)  # c b p n s
        a = a.transpose(3, 2, 0, 1, 4)             # n p c b s
        return np.ascontiguousarray(a)

    def pair_w(w8):
        # w8: [D, E] fp8 -> [p, c, b, E] with d = c*128 + b*64 + p
        a = w8.reshape(DC, 2, 64, E)  # c b p e
        a = a.transpose(2, 0, 1, 3)   # p c b e
        return np.ascontiguousarray(a)

    xT8 = {}
    xTv = {}
    for b in range(B):
        xT8[("q", b)] = pair_x2(
            np.ascontiguousarray(qi[b].T).astype(NP8))
        xT8[("k", b)] = pair_x2(
            np.ascontiguousarray(ki[b].T).astype(NP8))
        # v path: [n, p, dc, 512] fp16 with d = dc*128 + p
        a = np.ascontiguousarray(vi[b].T).astype(np.float16)
        a = a.reshape(DC, P, S // 512, 512).transpose(2, 1, 0, 3)
        xTv[b] = np.ascontiguousarray(a)

    in_maps = []
    for core in range(NCORES):
        b, hg = core // (NCORES // B), core % (NCORES // B)
        hs = slice(hg * HL, (hg + 1) * HL)
        wq_flat = np.transpose(W_Q[hs], (1, 0, 2)).reshape(D, E)
        wk_flat = np.transpose(W_K[hs], (1, 0, 2)).reshape(D, E)
        wv_flat = np.transpose(W_V[hs], (1, 0, 2)).reshape(D, E)
        in_maps.append({
            "xq8": xT8[("q", b)],
            "xk8": xT8[("k", b)],
            "xv": xTv[b],
            "wq8": pair_w((wq_flat * WSC).astype(NP8)),
            "wk8": pair_w((wk_flat * WSC).astype(NP8)),
            "wv": np.ascontiguousarray(
                wv_flat.reshape(DC, P, E).transpose(1, 0, 2)).astype(
                    np.float16),
            "wo": np.ascontiguousarray(
                W_O[hs].reshape(E, D).reshape(EC, P, D).transpose(
                    1, 0, 2)).astype(np.float16),
            "bq": np.ascontiguousarray(
                (b_Q[hs].reshape(E) * WSC).astype(np.float32)),
            "bk": np.ascontiguousarray(
                (b_K[hs].reshape(E) * WSC).astype(np.float32)),
            "bv": np.ascontiguousarray(b_V[hs].reshape(E)),
            "tri": tri128,
        })
    return in_maps


def gather_out(results, b_O):
    out = np.zeros((B, S, D), dtype=np.float64)
    for core in range(NCORES):
        out[core // (NCORES // B)] += results[core]["out"].astype(np.float64)
    out += np.asarray(b_O, dtype=np.float64)
    return out.astype(np.float32)


def kernel(query_input, key_input, value_input, W_Q, W_K, W_V, W_O,
           b_Q, b_K, b_V, b_O):
    nc = _get_nc()
    in_maps = make_in_maps(query_input, key_input, value_input,
                           W_Q, W_K, W_V, W_O, b_Q, b_K, b_V, b_O)
    res = run_bass_kernel_spmd(nc, in_maps, list(range(NCORES)))
    return gather_out(res.results, b_O)


def kernel_timed(inputs, trace_cores=None, **kwargs):
    """Like kernel() but traces and returns (out, BassKernelResults)."""
    nc = _get_nc()
    in_maps = make_in_maps(**inputs)
    res = run_bass_kernel_spmd(
        nc, in_maps, list(range(NCORES)), trace=True,
        trace_cores=trace_cores, **kwargs)
    return gather_out(res.results, inputs["b_O"]), res


# revision 26
# speedup vs baseline: 1.1595x; 1.0207x over previous
"""Trainium2 Bass kernel for multi-head causal attention.

Problem: B=2, S=2048, D=1024, H=16, DH=64 (fp32), causal attention with
QKV projections and output projection summed over heads.

Sharding: 8 cores = (batch b in {0,1}) x (head-group hg in {0..3}, 4 heads
each).  Each core computes a partial output sum over its 4 heads for its
batch; the host sums the 4 partials per batch and adds b_O.

v2 (fp8 DoubleRow): the Q/K projections and the score matmuls run in
fp8-e4m3 DoubleRow mode (0.5 cycles/row vs 1.0 for fp16), halving their
PE cost.  W_Q/W_K are pre-scaled by 64 on the host (keeps the fp8 values
out of the subnormal range), x_q/x_k are shipped fp8 in d-pair layout
[64p, 2, ...]; the projection PSUM result is written to SBUF as fp8
directly by DVE (scale 2^-5 + bias), then a cheap SBUF->SBUF DMA folds
the e-partition dim into (32 partitions, 2 pair columns) per head for the
DoubleRow score matmuls.  The leftover scale (64*2^-5)^2 = 4 is folded
into the Exp activation's scale together with 1/ATTN_SCALE (1/32 total).

V projection, PV, and the output projection stay fp16/f32r: emulation
shows fp8 there pushes rel err past the 2e-2 gate, while this config
measures ~1.1e-2 end to end.

The softmax denominator comes from 64 ones-columns appended to v (PSUM
partitions 64..127 of the PV matmul all hold l); normalization is a
single DVE divide.  Output partials are stored fp16 (halves the output
DMA) and summed on the host in float64.

A BIR post-processing patch (installed on import) hoists excess sync waits
off instructions into standalone EventSemaphore ops — walrus codegen allows
only 1 wait on the fused 4-byte-weight-load matmul encoding and few on
other opcodes, and Tile emits more.
"""

import sys

import numpy as np

for _p in ("/opt/trn_rl_repo",):
    if _p not in sys.path:
        sys.path.insert(0, _p)

import ml_dtypes

import concourse.bass as bass
import concourse.tile as tile
from concourse import mybir
from concourse.bass_utils import run_bass_kernel_spmd


def _hoist_matmul_waits(bir_json: bytes) -> bytes:
    """Move extra sync waits off instructions into EventSemaphore ops."""
    import orjson

    m = orjson.loads(bir_json)
    changed = False
    for fn in m.get("functions", []):
        for bb in fn.get("blocks", []):
            insts = bb.get("instructions", [])
            out = []
            for inst in insts:
                si = inst.get("sync_info") or {}
                waits = si.get("on_wait") or []
                if len(waits) > 1:
                    keep = waits[-1]
                    for wi, w in enumerate(waits[:-1]):
                        out.append({
                            "debug": inst.get("debug", 0),
                            "engine": inst["engine"],
                            "ins": [],
                            "name": f"{inst['name']}-hw{wi}",
                            "opcode": "EventSemaphore",
                            "outs": [],
                            "sync_info": {"on_update": [],
                                          "on_wait": [w]},
                        })
                    si["on_wait"] = [keep]
                    inst["sync_info"] = si
                    changed = True
                out.append(inst)
            bb["instructions"] = out
    if not changed:
        return bir_json
    return orjson.dumps(m)


def _install_bir_patch():
    from concourse import bass2jax as _b2j
    from concourse import bass_utils as _bu

    if getattr(_b2j, "_mm_wait_patch", False):
        return

    _orig = _bu.compile_bir_kernel

    def _patched(bir_json, tmpdir, neff_name="file.neff"):
        return _orig(_hoist_matmul_waits(bir_json), tmpdir, neff_name)

    _b2j.compile_bir_kernel = _patched
    _bu.compile_bir_kernel = _patched
    _b2j._mm_wait_patch = True


_install_bir_patch()

# Problem dims (hardcoded per harness contract).
B, S, D, H, DH = 2, 2048, 1024, 16, 64
ATTN_SCALE = 8.0
NCORES = 8
HL = H // (NCORES // B)  # 4 local heads per core
E = HL * DH              # 256 local head dims
P = 128
DC = D // P              # 8 contraction chunks
EC = E // P              # 2 e-chunks
NSB = S // P             # 16 s-blocks of 128
NI = 1024                # i-group width for score strips
NG = S // NI             # 2 i-groups
WSC = 64.0               # host-side W_Q/W_K scale (power of 2)
QSC = 2.0 ** -5          # q/k fp8 quantization scale
# exp argument scale: (WSC*QSC)^2 * ATTN_SCALE undone at the exp
ESC = 1.0 / ((WSC * QSC) ** 2 * ATTN_SCALE)
F32 = mybir.dt.float32
F32R = mybir.dt.float32r
F16 = mybir.dt.float16
F8 = mybir.dt.float8e4
AF = mybir.ActivationFunctionType
DR = mybir.MatmulPerfMode.DoubleRow
NP8 = ml_dtypes.float8_e4m3


def _round_f32r(arr):
    """Round an fp32 array to float32r (tfloat32) representable values."""
    from neuronxcc.starfish.support import dtype as nxd
    a = np.ascontiguousarray(np.asarray(arr, dtype=np.float32))
    return np.asarray(nxd.static_cast(a, dtype=nxd.float32r)).view(np.float32)


NPRE = 2  # g=1 j-blocks whose scores+exp are front-loaded


def _emit(ctx, tc, xq8, xk8, xv, wq8, wk8, wv, wo, bq, bk, bv, tri, out):
    nc = tc.nc

    persist = ctx.enter_context(tc.tile_pool(name="persist", bufs=1))
    xstage = ctx.enter_context(tc.tile_pool(name="xstage", bufs=2))
    xvstage = ctx.enter_context(tc.tile_pool(name="xvstage", bufs=2))
    ptpool = ctx.enter_context(tc.tile_pool(name="ptp", bufs=6))
    outpool = ctx.enter_context(tc.tile_pool(name="outp", bufs=4))
    small = ctx.enter_context(tc.tile_pool(name="small", bufs=2))
    # PSUM budget (8 banks of [128, 2KB]):
    #   ps_s: score strips [128, 1024] = 2 banks x 2 bufs = 4
    #   ps_mm: proj / outproj [128, <=512] = 1 bank x 2 bufs = 2
    #   ps_z: PV accumulators [128, 512] = 1 bank x 2 bufs = 2
    ps_s = ctx.enter_context(tc.tile_pool(name="ps_s", bufs=2, space="PSUM"))
    ps_mm = ctx.enter_context(tc.tile_pool(name="ps_mm", bufs=2, space="PSUM"))
    ps_z = ctx.enter_context(tc.tile_pool(name="ps_z", bufs=2, space="PSUM"))

    # --- persistent activations ---
    # q/k in fp8, raw projection layout [e-part, ec, s]
    q8raw = persist.tile([P, EC, S], F8)
    k8raw = persist.tile([P, EC, S], F8)
    # pair-folded for DoubleRow scores: per e-chunk hc a [64, 2, S] tile,
    # head 2hc+j at partitions j*32..j*32+32 (AP base must be in {0,32,64})
    q8t = [persist.tile([64, 2, S], F8, name=f"q8t{hc}") for hc in range(EC)]
    k8t = [persist.tile([64, 2, S], F8, name=f"k8t{hc}") for hc in range(EC)]
    zT_sb = persist.tile([P, EC, S], F16)  # normalized z^T
    # v natural layout + 64 ones columns (rows 64..127 of PV psum become l)
    v_g = [persist.tile([P, NSB // NG, HL, 2 * DH], F16, name=f"v{g}")
           for g in range(NG)]
    # front-loaded pattern tiles (g0 fully, g1 j-blocks < NPRE)
    pt_g0 = [[persist.tile([P, NI], F16, name=f"pt0_{h}_{jb}")
              for jb in range(NI // P)] for h in range(HL)]
    pt_g1e = [[persist.tile([P, NI], F16, name=f"pt1_{h}_{jb}")
               for jb in range(NPRE)] for h in range(HL)]

    wk_sb = persist.tile([64, DC, 2, E], F8)
    wq_sb = persist.tile([64, DC, 2, E], F8)
    wv_sb = persist.tile([P, DC, E], F16)
    wo_sb = persist.tile([P, EC, D], F16)
    bq_sb = persist.tile([P, EC], F32)
    bk_sb = persist.tile([P, EC], F32)
    bv_bc = persist.tile([P, E], F32)
    tri_sb = persist.tile([P, P], F16)

    def emit_kq(g):
        if g == 0:
            # weights on the gpsimd DMA queue so they land in parallel with
            # the first x chunk on SP
            nc.gpsimd.dma_start(out=wk_sb, in_=wk8)
            nc.gpsimd.dma_start(out=wq_sb, in_=wq8)
        for nl in range(NI // 512):  # local 512-col chunks
            n = g * (NI // 512) + nl
            for x8_r, w_sb, b_sb, raw, dst in (
                (xk8, wk_sb, bk_sb, k8raw, k8t),
                (xq8, wq_sb, bq_sb, q8raw, q8t),
            ):
                xs = xstage.tile([64, DC, 2, 512], F8, tag="xs")
                nc.sync.dma_start(out=xs, in_=x8_r[n])
                if g == 0 and nl == 0:
                    nc.sync.dma_start(
                        out=(bk_sb if raw is k8raw else bq_sb),
                        in_=(bk if raw is k8raw else bq).rearrange(
                            "(c p) -> p c", p=P))
                for m in range(EC):
                    ps = ps_mm.tile([P, 512], F32, tag="mm")
                    for c in range(DC):
                        nc.tensor.matmul(
                            ps,
                            lhsT=w_sb[:, c, :, m * P:(m + 1) * P],
                            rhs=xs[:, c, :, :],
                            start=(c == 0),
                            stop=(c == DC - 1),
                            perf_mode=DR,
                        )
                    # raw = (ps + bias) * 2^-5, quantized to fp8
                    nc.vector.tensor_scalar(
                        out=raw[:, m, n * 512:(n + 1) * 512],
                        in0=ps,
                        scalar1=b_sb[:, m:m + 1],
                        scalar2=QSC,
                        op0=mybir.AluOpType.add,
                        op1=mybir.AluOpType.mult,
                    )
        # fold e-partitions into (32p, 2) pairs for DoubleRow: partition p ->
        # out partition p//2, pair col p%2 (heads 2m, 2m+1 at bases 0, 32 of
        # tile m).  g=0 folds ride the idle Activation hwdge queue so they
        # don't head-of-line-block the g=1 x loads on SP.
        eng = nc.scalar if g == 0 else nc.sync
        for m in range(EC):
            for raw, dst in ((k8raw, k8t), (q8raw, q8t)):
                for nl in range(NI // 512):
                    n = g * (NI // 512) + nl
                    eng.dma_start(
                        out=dst[m][:, :, n * 512:(n + 1) * 512],
                        in_=raw[:, m, n * 512:(n + 1) * 512],
                    )

    def emit_v(g):
        # ones columns for the whole group in one memset (fp16 is legal)
        nc.vector.memset(v_g[g][:, :, :, DH:2 * DH], 1.0)
        for nl in range(NI // 512):  # one xv chunk per 512 cols
            n = g * (NI // 512) + nl
            xs = xvstage.tile([P, DC, 512], F16, tag="xv")
            nc.gpsimd.dma_start(out=xs, in_=xv[n])
            for sl in range(512 // P):
                sbl = nl * (512 // P) + sl
                ps = ps_mm.tile([P, E], F32, tag="mm")
                for dc in range(DC):
                    nc.tensor.matmul(
                        ps,
                        lhsT=xs[:, dc, sl * P:(sl + 1) * P],
                        rhs=wv_sb[:, dc, :],
                        start=(dc == 0),
                        stop=(dc == DC - 1),
                    )
                nc.vector.tensor_add(
                    out=v_g[g][:, sbl, :, 0:DH],
                    in0=ps.rearrange("p (h e) -> p h e", h=HL),
                    in1=bv_bc.rearrange("p (h e) -> p h e", h=HL),
                )

    def emit_scores(g, jbs, pt_dst):
        """Scores + exp (+ causal mask) for strips of group g into pt_dst."""
        for h in range(HL):
            hc, hbit = h // 2, h % 2
            hb = hbit * 32
            for jb in jbs:
                t = jb - (NI // P) * g
                ct = 0 if t < 4 else 1
                sps = ps_s.tile([P, NI], F32, tag="s")
                pt = pt_dst(h, jb)
                zlo = max(0, t) * P
                for c in range(ct, 2):
                    c0 = c * 512
                    lo = max(zlo, c0)
                    nc.tensor.matmul(
                        sps[:, lo:c0 + 512],
                        lhsT=k8t[hc][hb:hb + 32, :, jb * P:(jb + 1) * P],
                        rhs=q8t[hc][hb:hb + 32, :,
                                    g * NI + lo:g * NI + c0 + 512],
                        start=True,
                        stop=True,
                        perf_mode=DR,
                    )
                nc.scalar.activation(out=pt[:, zlo:NI],
                                     in_=sps[:, zlo:NI], func=AF.Exp,
                                     scale=ESC)
                if t >= 0:
                    # triangle mask on the diagonal 128 columns (GpSimd:
                    # SBUF-only op, keeps DVE free)
                    nc.gpsimd.tensor_mul(
                        out=pt[:, zlo:zlo + P],
                        in0=pt[:, zlo:zlo + P],
                        in1=tri_sb,
                    )

    def emit_pv_norm(g, pt_src):
        """PV + normalization for all heads of group g.

        pt_src(h, jb) returns the pattern tile, or None if its scores+exp
        still need to be emitted here (tail of g=1).
        """
        jmax = (NI // P) * g + (NI // P)
        for h in range(HL):
            hc, hbit = h // 2, h % 2
            hb = hbit * 32
            e0 = hbit * DH

            def _ct(jb):
                t = jb - (NI // P) * g
                return 0 if t < 4 else 1

            contrib = [[jb for jb in range(jmax) if _ct(jb) <= c]
                       for c in range(2)]
            zps = [ps_z.tile([2 * DH, 512], F32, tag="z", name=f"zps{c}")
                   for c in range(2)]
            for jb in range(jmax):
                t = jb - (NI // P) * g
                ct = _ct(jb)
                zlo = max(0, t) * P
                pt = pt_src(h, jb)
                if pt is None:
                    sps = ps_s.tile([P, NI], F32, tag="s")
                    pt = ptpool.tile([P, NI], F16, tag="pt")
                    for c in range(ct, 2):
                        c0 = c * 512
                        lo = max(zlo, c0)
                        nc.tensor.matmul(
                            sps[:, lo:c0 + 512],
                            lhsT=k8t[hc][hb:hb + 32, :,
                                         jb * P:(jb + 1) * P],
                            rhs=q8t[hc][hb:hb + 32, :,
                                        g * NI + lo:g * NI + c0 + 512],
                            start=True,
                            stop=True,
                            perf_mode=DR,
                        )
                    nc.scalar.activation(out=pt[:, zlo:NI],
                                         in_=sps[:, zlo:NI], func=AF.Exp,
                                         scale=ESC)
                    if t >= 0:
                        nc.gpsimd.tensor_mul(
                            out=pt[:, zlo:zlo + P],
                            in0=pt[:, zlo:zlo + P],
                            in1=tri_sb,
                        )
                for c in range(ct, 2):
                    c0 = c * 512
                    lo = max(zlo, c0)  # masked cols are simply never read
                    nc.tensor.matmul(
                        zps[c][:, lo - c0:512],
                        lhsT=v_g[jb // (NSB // NG)][
                            :, jb % (NSB // NG), h, :],
                        rhs=pt[:, lo:c0 + 512],
                        start=(jb == contrib[c][0]),
                        stop=(jb == contrib[c][-1]),
                    )
                    if jb == contrib[c][-1]:
                        # normalize as soon as this c-chunk's accumulation
                        # completes: zT = z * (1/l); rows DH..2DH hold l
                        bcr = small.tile([DH, 512], F32, tag="bcr")
                        nc.vector.reciprocal(bcr, zps[c][DH:2 * DH, :])
                        icol = g * NI + c * 512
                        nc.vector.tensor_mul(
                            out=zT_sb[e0:e0 + DH, hc, icol:icol + 512],
                            in0=zps[c][0:DH, :],
                            in1=bcr,
                        )

    def emit_outproj(g):
        for ib in range((NI // P) * g, (NI // P) * g + NI // P):
            osb = outpool.tile([P, D], F16, tag="o")
            for d2 in range(2):
                ops = ps_mm.tile([P, 512], F32, tag="mm")
                for ec in range(EC):
                    nc.tensor.matmul(
                        ops,
                        lhsT=zT_sb[:, ec, ib * P:(ib + 1) * P],
                        rhs=wo_sb[:, ec, d2 * 512:(d2 + 1) * 512],
                        start=(ec == 0),
                        stop=(ec == EC - 1),
                    )
                nc.vector.tensor_copy(
                    out=osb[:, d2 * 512:(d2 + 1) * 512], in_=ops)
            eng = nc.gpsimd if ib % 2 == 0 else nc.sync
            eng.dma_start(out=out[ib * P:(ib + 1) * P, :], in_=osb)

    # secondary loads early on the gpsimd queue (wk/wq go first inside
    # emit_kq(0) below)
    def emit_v_loads(g):
        if g == 0:
            nc.gpsimd.dma_start(out=wv_sb, in_=wv)
            nc.gpsimd.dma_start(out=tri_sb, in_=tri)
            bv_bcast_ap = bass.AP(tensor=bv.tensor, offset=bv.offset,
                                  ap=[[0, P]] + list(bv.ap))
            nc.gpsimd.dma_start(out=bv_bc, in_=bv_bcast_ap)
        else:
            nc.gpsimd.dma_start(out=wo_sb, in_=wo)

    emit_kq(0)
    emit_v_loads(0)
    emit_scores(0, range(NI // P), lambda h, jb: pt_g0[h][jb])
    emit_kq(1)
    emit_v_loads(1)
    emit_scores(1, range(NPRE), lambda h, jb: pt_g1e[h][jb])
    emit_v(0)
    emit_pv_norm(0, lambda h, jb: pt_g0[h][jb])
    emit_v(1)
    emit_outproj(0)
    emit_pv_norm(1, lambda h, jb: pt_g1e[h][jb] if jb < NPRE else None)
    emit_outproj(1)


def build_nc():
    from contextlib import ExitStack

    nc = bass.Bass()
    # x for q/k: fp8 in DoubleRow d-pair layout, chunked by 512 s-columns:
    # [n, p, c, b, s] with d = c*128 + b*64 + p (64 partitions, base 0)
    xq8 = nc.dram_tensor("xq8", [S // 512, 64, DC, 2, 512], F8,
                         kind="ExternalInput")[:]
    xk8 = nc.dram_tensor("xk8", [S // 512, 64, DC, 2, 512], F8,
                         kind="ExternalInput")[:]
    # x for v: fp16 [n, p, dc, s] with d = dc*128 + p
    xv = nc.dram_tensor("xv", [S // 512, P, DC, 512], F16,
                        kind="ExternalInput")[:]
    # w for q/k: fp8 pre-scaled by WSC, same d-pair layout
    wq8 = nc.dram_tensor("wq8", [64, DC, 2, E], F8,
                         kind="ExternalInput")[:]
    wk8 = nc.dram_tensor("wk8", [64, DC, 2, E], F8,
                         kind="ExternalInput")[:]
    wv = nc.dram_tensor("wv", [P, DC, E], F16, kind="ExternalInput")[:]
    wo = nc.dram_tensor("wo", [P, EC, D], F16, kind="ExternalInput")[:]
    bq = nc.dram_tensor("bq", [E], F32, kind="ExternalInput")[:]
    bk = nc.dram_tensor("bk", [E], F32, kind="ExternalInput")[:]
    bv = nc.dram_tensor("bv", [E], F32, kind="ExternalInput")[:]
    tri = nc.dram_tensor("tri", [P, P], F16, kind="ExternalInput")[:]
    out = nc.dram_tensor("out", [S, D], F16, kind="ExternalOutput")[:]
    with tile.TileContext(nc) as tc:
        with ExitStack() as ctx:
            _emit(ctx, tc, xq8, xk8, xv, wq8, wk8, wv, wo, bq, bk, bv, tri,
                  out)
    return nc


_CACHE = {}


def _get_nc():
    if "nc" not in _CACHE:
        _CACHE["nc"] = build_nc()
    return _CACHE["nc"]


def make_in_maps(query_input, key_input, value_input, W_Q, W_K, W_V, W_O,
                 b_Q, b_K, b_V, b_O):
    qi = np.asarray(query_input, dtype=np.float32)
    ki = np.asarray(key_input, dtype=np.float32)
    vi = np.asarray(value_input, dtype=np.float32)
    W_Q = np.asarray(W_Q, dtype=np.float32)
    W_K = np.asarray(W_K, dtype=np.float32)
    W_V = np.asarray(W_V, dtype=np.float32)
    W_O = np.asarray(W_O, dtype=np.float32)
    b_Q = np.asarray(b_Q, dtype=np.float32)
    b_K = np.asarray(b_K, dtype=np.float32)
    b_V = np.asarray(b_V, dtype=np.float32)

    tri128 = np.triu(np.ones((P, P), dtype=np.float16))  # tri[j, i] = i >= j

    def pair_x2(xT8):
        # xT8: [D, S] fp8 -> [n, p, c, b, 512] with d = c*128 + b*64 + p
        a = xT8.reshape(DC, 2, 64, S // 512, 512)  # c b p n s
        a = a.transpose(3, 2, 0, 1, 4)             # n p c b s
        return np.ascontiguousarray(a)

    def pair_w(w8):
        # w8: [D, E] fp8 -> [p, c, b, E] with d = c*128 + b*64 + p
        a = w8.reshape(DC, 2, 64, E)  # c b p e
        a = a.transpose(2, 0, 1, 3)   # p c b e
        return np.ascontiguousarray(a)

    xT8 = {}
    xTv = {}
    for b in range(B):
        xT8[("q", b)] = pair_x2(
            np.ascontiguousarray(qi[b].T).astype(NP8))
        xT8[("k", b)] = pair_x2(
            np.ascontiguousarray(ki[b].T).astype(NP8))
        # v path: [n, p, dc, 512] fp16 with d = dc*128 + p
        a = np.ascontiguousarray(vi[b].T).astype(np.float16)
        a = a.reshape(DC, P, S // 512, 512).transpose(2, 1, 0, 3)
        xTv[b] = np.ascontiguousarray(a)

    in_maps = []
    for core in range(NCORES):
        b, hg = core // (NCORES // B), core % (NCORES // B)
        hs = slice(hg * HL, (hg + 1) * HL)
        wq_flat = np.transpose(W_Q[hs], (1, 0, 2)).reshape(D, E)
        wk_flat = np.transpose(W_K[hs], (1, 0, 2)).reshape(D, E)
        wv_flat = np.transpose(W_V[hs], (1, 0, 2)).reshape(D, E)
        in_maps.append({
            "xq8": xT8[("q", b)],
            "xk8": xT8[("k", b)],
            "xv": xTv[b],
            "wq8": pair_w((wq_flat * WSC).astype(NP8)),
            "wk8": pair_w((wk_flat * WSC).astype(NP8)),
            "wv": np.ascontiguousarray(
                wv_flat.reshape(DC, P, E).transpose(1, 0, 2)).astype(
                    np.float16),
            "wo": np.ascontiguousarray(
                W_O[hs].reshape(E, D).reshape(EC, P, D).transpose(
                    1, 0, 2)).astype(np.float16),
            "bq": np.ascontiguousarray(
                (b_Q[hs].reshape(E) * WSC).astype(np.float32)),
            "bk": np.ascontiguousarray(
                (b_K[hs].reshape(E) * WSC).astype(np.float32)),
            "bv": np.ascontiguousarray(b_V[hs].reshape(E)),
            "tri": tri128,
        })
    return in_maps


def gather_out(results, b_O):
    out = np.zeros((B, S, D), dtype=np.float64)
    for core in range(NCORES):
        out[core // (NCORES // B)] += results[core]["out"].astype(np.float64)
    out += np.asarray(b_O, dtype=np.float64)
    return out.astype(np.float32)


def kernel(query_input, key_input, value_input, W_Q, W_K, W_V, W_O,
           b_Q, b_K, b_V, b_O):
    nc = _get_nc()
    in_maps = make_in_maps(query_input, key_input, value_input,
                           W_Q, W_K, W_V, W_O, b_Q, b_K, b_V, b_O)
    res = run_bass_kernel_spmd(nc, in_maps, list(range(NCORES)))
    return gather_out(res.results, b_O)


def kernel_timed(inputs, trace_cores=None, **kwargs):
    """Like kernel() but traces and returns (out, BassKernelResults)."""
    nc = _get_nc()
    in_maps = make_in_maps(**inputs)
    res = run_bass_kernel_spmd(
        nc, in_maps, list(range(NCORES)), trace=True,
        trace_cores=trace_cores, **kwargs)
    return gather_out(res.results, inputs["b_O"]), res


# revision 33
# speedup vs baseline: 1.1658x; 1.0054x over previous
"""Trainium2 Bass kernel for multi-head causal attention.

Problem: B=2, S=2048, D=1024, H=16, DH=64 (fp32), causal attention with
QKV projections and output projection summed over heads.

Sharding: 8 cores = (batch b in {0,1}) x (head-group hg in {0..3}, 4 heads
each).  Each core computes a partial output sum over its 4 heads for its
batch; the host sums the 4 partials per batch and adds b_O.

v2 (fp8 DoubleRow): the Q/K projections and the score matmuls run in
fp8-e4m3 DoubleRow mode (0.5 cycles/row vs 1.0 for fp16), halving their
PE cost.  W_Q/W_K are pre-scaled by 64 on the host (keeps the fp8 values
out of the subnormal range), x_q/x_k are shipped fp8 in d-pair layout
[64p, 2, ...]; the projection PSUM result is written to SBUF as fp8
directly by DVE (scale 2^-5 + bias), then a cheap SBUF->SBUF DMA folds
the e-partition dim into (32 partitions, 2 pair columns) per head for the
DoubleRow score matmuls.  The leftover scale (64*2^-5)^2 = 4 is folded
into the Exp activation's scale together with 1/ATTN_SCALE (1/32 total).

V projection, PV, and the output projection stay fp16/f32r: emulation
shows fp8 there pushes rel err past the 2e-2 gate, while this config
measures ~1.1e-2 end to end.

The softmax denominator comes from 64 ones-columns appended to v (PSUM
partitions 64..127 of the PV matmul all hold l); normalization is a
single DVE divide.  Output partials are stored fp16 (halves the output
DMA) and summed on the host in float64.

A BIR post-processing patch (installed on import) hoists excess sync waits
off instructions into standalone EventSemaphore ops — walrus codegen allows
only 1 wait on the fused 4-byte-weight-load matmul encoding and few on
other opcodes, and Tile emits more.
"""

import sys

import numpy as np

for _p in ("/opt/trn_rl_repo",):
    if _p not in sys.path:
        sys.path.insert(0, _p)

import ml_dtypes

import concourse.bass as bass
import concourse.tile as tile
from concourse import mybir
from concourse.bass_utils import run_bass_kernel_spmd


def _hoist_matmul_waits(bir_json: bytes) -> bytes:
    """Move extra sync waits off instructions into EventSemaphore ops."""
    import orjson

    m = orjson.loads(bir_json)
    changed = False
    for fn in m.get("functions", []):
        for bb in fn.get("blocks", []):
            insts = bb.get("instructions", [])
            out = []
            for inst in insts:
                si = inst.get("sync_info") or {}
                waits = si.get("on_wait") or []
                if len(waits) > 1:
                    keep = waits[-1]
                    for wi, w in enumerate(waits[:-1]):
                        out.append({
                            "debug": inst.get("debug", 0),
                            "engine": inst["engine"],
                            "ins": [],
                            "name": f"{inst['name']}-hw{wi}",
                            "opcode": "EventSemaphore",
                            "outs": [],
                            "sync_info": {"on_update": [],
                                          "on_wait": [w]},
                        })
                    si["on_wait"] = [keep]
                    inst["sync_info"] = si
                    changed = True
                out.append(inst)
            bb["instructions"] = out
    if not changed:
        return bir_json
    return orjson.dumps(m)


def _install_bir_patch():
    from concourse import bass2jax as _b2j
    from concourse import bass_utils as _bu

    if getattr(_b2j, "_mm_wait_patch", False):
        return

    _orig = _bu.compile_bir_kernel

    def _patched(bir_json, tmpdir, neff_name="file.neff"):
        return _orig(_hoist_matmul_waits(bir_json), tmpdir, neff_name)

    _b2j.compile_bir_kernel = _patched
    _bu.compile_bir_kernel = _patched
    _b2j._mm_wait_patch = True


_install_bir_patch()

# Problem dims (hardcoded per harness contract).
B, S, D, H, DH = 2, 2048, 1024, 16, 64
ATTN_SCALE = 8.0
NCORES = 8
HL = H // (NCORES // B)  # 4 local heads per core
E = HL * DH              # 256 local head dims
P = 128
DC = D // P              # 8 contraction chunks
EC = E // P              # 2 e-chunks
NSB = S // P             # 16 s-blocks of 128
NI = 1024                # i-group width for score strips
NG = S // NI             # 2 i-groups
WSC = 64.0               # host-side W_Q/W_K scale (power of 2)
QSC = 2.0 ** -5          # q/k fp8 quantization scale
# exp argument scale: (WSC*QSC)^2 * ATTN_SCALE undone at the exp
ESC = 1.0 / ((WSC * QSC) ** 2 * ATTN_SCALE)
F32 = mybir.dt.float32
F32R = mybir.dt.float32r
F16 = mybir.dt.float16
F8 = mybir.dt.float8e4
AF = mybir.ActivationFunctionType
DR = mybir.MatmulPerfMode.DoubleRow
NP8 = ml_dtypes.float8_e4m3


def _round_f32r(arr):
    """Round an fp32 array to float32r (tfloat32) representable values."""
    from neuronxcc.starfish.support import dtype as nxd
    a = np.ascontiguousarray(np.asarray(arr, dtype=np.float32))
    return np.asarray(nxd.static_cast(a, dtype=nxd.float32r)).view(np.float32)


# g=1 strips whose scores+exp are front-loaded (computed during the
# projection/PV-g0 phases so the Activation engine never idles): all of h3,
# the first 8 j-blocks of h2.  g=0 is always fully front-loaded.
PRE_G1 = {3: range(16), 2: range(8)}


def _emit(ctx, tc, xq8, xk8, xv, wq8, wk8, wv, wo, bq, bk, bv, tri, out):
    nc = tc.nc

    persist = ctx.enter_context(tc.tile_pool(name="persist", bufs=1))
    xstage = ctx.enter_context(tc.tile_pool(name="xstage", bufs=2))
    xvstage = ctx.enter_context(tc.tile_pool(name="xvstage", bufs=2))
    ptpool = ctx.enter_context(tc.tile_pool(name="ptp", bufs=6))
    outpool = ctx.enter_context(tc.tile_pool(name="outp", bufs=4))
    small = ctx.enter_context(tc.tile_pool(name="small", bufs=2))
    # PSUM budget (8 banks of [128, 2KB]):
    #   ps_s: score strips [128, 1024] = 2 banks x 2 bufs = 4
    #   ps_mm: proj / outproj [128, <=512] = 1 bank x 2 bufs = 2
    #   ps_z: PV accumulators [128, 512] = 1 bank x 2 bufs = 2
    ps_s = ctx.enter_context(tc.tile_pool(name="ps_s", bufs=2, space="PSUM"))
    ps_mm = ctx.enter_context(tc.tile_pool(name="ps_mm", bufs=2, space="PSUM"))
    ps_z = ctx.enter_context(tc.tile_pool(name="ps_z", bufs=2, space="PSUM"))

    # --- persistent activations ---
    # q/k in fp8, raw projection layout [e-part, ec, s]
    q8raw = persist.tile([P, EC, S], F8)
    k8raw = persist.tile([P, EC, S], F8)
    # pair-folded for DoubleRow scores: per e-chunk hc a [64, 2, S] tile,
    # head 2hc+j at partitions j*32..j*32+32 (AP base must be in {0,32,64})
    q8t = [persist.tile([64, 2, S], F8, name=f"q8t{hc}") for hc in range(EC)]
    k8t = [persist.tile([64, 2, S], F8, name=f"k8t{hc}") for hc in range(EC)]
    zT_sb = persist.tile([P, EC, S], F16)  # normalized z^T
    # v natural layout + 64 ones columns (rows 64..127 of PV psum become l)
    v_g = [persist.tile([P, NSB // NG, HL, 2 * DH], F16, name=f"v{g}")
           for g in range(NG)]
    # front-loaded pattern tiles, exact causal width; value = (tile, zlo)
    pre_pt = {}
    for h in range(HL):
        for jb in range(NI // P):
            zlo = jb * P
            pre_pt[(0, h, jb)] = (
                persist.tile([P, NI - zlo], F16, name=f"pt0_{h}_{jb}"), zlo)
    for h, jbs in PRE_G1.items():
        for jb in jbs:
            zlo = max(0, jb - NI // P) * P
            pre_pt[(1, h, jb)] = (
                persist.tile([P, NI - zlo], F16, name=f"pt1_{h}_{jb}"), zlo)

    wk_sb = persist.tile([64, DC, 2, E], F8)
    wq_sb = persist.tile([64, DC, 2, E], F8)
    wv_sb = persist.tile([P, DC, E], F16)
    wo_sb = persist.tile([P, EC, D], F16)
    bq_sb = persist.tile([P, EC], F32)
    bk_sb = persist.tile([P, EC], F32)
    bv_bc = persist.tile([P, E], F32)
    tri_sb = persist.tile([P, P], F16)

    def emit_kq(g):
        if g == 0:
            # weights on the gpsimd DMA queue so they land in parallel with
            # the first x chunk on SP
            nc.gpsimd.dma_start(out=wk_sb, in_=wk8)
            nc.gpsimd.dma_start(out=wq_sb, in_=wq8)
        for nl in range(NI // 512):  # local 512-col chunks
            n = g * (NI // 512) + nl
            for x8_r, w_sb, b_sb, raw, dst in (
                (xq8, wq_sb, bq_sb, q8raw, q8t),
                (xk8, wk_sb, bk_sb, k8raw, k8t),
            ):
                xs = xstage.tile([64, DC, 2, 512], F8, tag="xs")
                nc.sync.dma_start(out=xs, in_=x8_r[n])
                if g == 0 and nl == 0:
                    nc.sync.dma_start(
                        out=(bk_sb if raw is k8raw else bq_sb),
                        in_=(bk if raw is k8raw else bq).rearrange(
                            "(c p) -> p c", p=P))
                for m in range(EC):
                    ps = ps_mm.tile([P, 512], F32, tag="mm")
                    for c in range(DC):
                        nc.tensor.matmul(
                            ps,
                            lhsT=w_sb[:, c, :, m * P:(m + 1) * P],
                            rhs=xs[:, c, :, :],
                            start=(c == 0),
                            stop=(c == DC - 1),
                            perf_mode=DR,
                        )
                    # raw = (ps + bias) * 2^-5, quantized to fp8
                    nc.vector.tensor_scalar(
                        out=raw[:, m, n * 512:(n + 1) * 512],
                        in0=ps,
                        scalar1=b_sb[:, m:m + 1],
                        scalar2=QSC,
                        op0=mybir.AluOpType.add,
                        op1=mybir.AluOpType.mult,
                    )
        # fold e-partitions into (32p, 2) pairs for DoubleRow: partition p ->
        # out partition p//2, pair col p%2 (heads 2m, 2m+1 at bases 0, 32 of
        # tile m).  g=0 folds ride the idle Activation hwdge queue so they
        # don't head-of-line-block the g=1 x loads on SP; fold order q/k
        # alternating, m=0 first, so head 0's operands land earliest.
        eng = nc.scalar if g == 0 else nc.sync
        for m in range(EC):
            for nl in range(NI // 512):
                n = g * (NI // 512) + nl
                for raw, dst in ((q8raw, q8t), (k8raw, k8t)):
                    eng.dma_start(
                        out=dst[m][:, :, n * 512:(n + 1) * 512],
                        in_=raw[:, m, n * 512:(n + 1) * 512],
                    )

    def emit_v(g):
        # ones columns for the whole group in one memset (fp16 is legal)
        nc.vector.memset(v_g[g][:, :, :, DH:2 * DH], 1.0)
        for nl in range(NI // 512):  # one xv chunk per 512 cols
            n = g * (NI // 512) + nl
            xs = xvstage.tile([P, DC, 512], F16, tag="xv")
            nc.sync.dma_start(out=xs, in_=xv[n])
            for sl in range(512 // P):
                sbl = nl * (512 // P) + sl
                ps = ps_mm.tile([P, E], F32, tag="mm")
                for dc in range(DC):
                    nc.tensor.matmul(
                        ps,
                        lhsT=xs[:, dc, sl * P:(sl + 1) * P],
                        rhs=wv_sb[:, dc, :],
                        start=(dc == 0),
                        stop=(dc == DC - 1),
                    )
                nc.vector.tensor_add(
                    out=v_g[g][:, sbl, :, 0:DH],
                    in0=ps.rearrange("p (h e) -> p h e", h=HL),
                    in1=bv_bc.rearrange("p (h e) -> p h e", h=HL),
                )

    def emit_strip(g, h, jb, pt, off):
        """Scores + exp (+ causal mask) for one strip; pt covers columns
        [off, NI) of the strip's i-range."""
        hc, hbit = h // 2, h % 2
        hb = hbit * 32
        t = jb - (NI // P) * g
        ct = 0 if t < 4 else 1
        zlo = max(0, t) * P
        sps = ps_s.tile([P, NI], F32, tag="s")
        for c in range(ct, 2):
            c0 = c * 512
            lo = max(zlo, c0)
            nc.tensor.matmul(
                sps[:, lo:c0 + 512],
                lhsT=k8t[hc][hb:hb + 32, :, jb * P:(jb + 1) * P],
                rhs=q8t[hc][hb:hb + 32, :,
                            g * NI + lo:g * NI + c0 + 512],
                start=True,
                stop=True,
                perf_mode=DR,
            )
        nc.scalar.activation(out=pt[:, zlo - off:NI - off],
                             in_=sps[:, zlo:NI], func=AF.Exp,
                             scale=ESC)
        if t >= 0:
            # triangle mask on the diagonal 128 columns (GpSimd:
            # SBUF-only op, keeps DVE free)
            nc.gpsimd.tensor_mul(
                out=pt[:, zlo - off:zlo - off + P],
                in0=pt[:, zlo - off:zlo - off + P],
                in1=tri_sb,
            )

    def emit_pre(g, hs):
        for h in hs:
            for jb in (j for j in range(16) if (g, h, j) in pre_pt):
                pt, off = pre_pt[(g, h, jb)]
                emit_strip(g, h, jb, pt, off)

    def emit_pv_norm(g, head_order):
        """PV + normalization for the heads of group g (inline scores+exp
        for strips not in pre_pt)."""
        jmax = (NI // P) * g + (NI // P)
        for h in head_order:
            hc, hbit = h // 2, h % 2
            e0 = hbit * DH

            def _ct(jb):
                t = jb - (NI // P) * g
                return 0 if t < 4 else 1

            contrib = [[jb for jb in range(jmax) if _ct(jb) <= c]
                       for c in range(2)]
            zps = [ps_z.tile([2 * DH, 512], F32, tag="z", name=f"zps{c}")
                   for c in range(2)]
            for jb in range(jmax):
                t = jb - (NI // P) * g
                ct = _ct(jb)
                zlo = max(0, t) * P
                if (g, h, jb) in pre_pt:
                    pt, off = pre_pt[(g, h, jb)]
                else:
                    pt = ptpool.tile([P, NI], F16, tag="pt")
                    off = 0
                    emit_strip(g, h, jb, pt, off)
                for c in range(ct, 2):
                    c0 = c * 512
                    lo = max(zlo, c0)  # masked cols are simply never read
                    nc.tensor.matmul(
                        zps[c][:, lo - c0:512],
                        lhsT=v_g[jb // (NSB // NG)][
                            :, jb % (NSB // NG), h, :],
                        rhs=pt[:, lo - off:c0 + 512 - off],
                        start=(jb == contrib[c][0]),
                        stop=(jb == contrib[c][-1]),
                    )
                    if jb == contrib[c][-1]:
                        # normalize as soon as this c-chunk's accumulation
                        # completes: zT = z * (1/l); rows DH..2DH hold l
                        bcr = small.tile([DH, 512], F32, tag="bcr")
                        nc.vector.reciprocal(bcr, zps[c][DH:2 * DH, :])
                        icol = g * NI + c * 512
                        nc.vector.tensor_mul(
                            out=zT_sb[e0:e0 + DH, hc, icol:icol + 512],
                            in0=zps[c][0:DH, :],
                            in1=bcr,
                        )

    def emit_outproj(g):
        for ib in range((NI // P) * g, (NI // P) * g + NI // P):
            osb = outpool.tile([P, D], F16, tag="o")
            for d2 in range(2):
                ops = ps_mm.tile([P, 512], F32, tag="mm")
                for ec in range(EC):
                    nc.tensor.matmul(
                        ops,
                        lhsT=zT_sb[:, ec, ib * P:(ib + 1) * P],
                        rhs=wo_sb[:, ec, d2 * 512:(d2 + 1) * 512],
                        start=(ec == 0),
                        stop=(ec == EC - 1),
                    )
                nc.vector.tensor_copy(
                    out=osb[:, d2 * 512:(d2 + 1) * 512], in_=ops)
            eng = nc.gpsimd if ib % 2 == 0 else nc.sync
            eng.dma_start(out=out[ib * P:(ib + 1) * P, :], in_=osb)

    # secondary loads early on the gpsimd queue (wk/wq go first inside
    # emit_kq(0) below)
    def emit_v_loads(g):
        if g == 0:
            nc.gpsimd.dma_start(out=wv_sb, in_=wv)
            nc.gpsimd.dma_start(out=tri_sb, in_=tri)
            bv_bcast_ap = bass.AP(tensor=bv.tensor, offset=bv.offset,
                                  ap=[[0, P]] + list(bv.ap))
            nc.gpsimd.dma_start(out=bv_bc, in_=bv_bcast_ap)
        else:
            nc.gpsimd.dma_start(out=wo_sb, in_=wo)

    emit_kq(0)
    emit_v_loads(0)
    emit_pre(0, range(HL))
    emit_kq(1)
    emit_v_loads(1)
    emit_pre(1, sorted(PRE_G1))
    emit_v(0)
    emit_pv_norm(0, range(HL))
    emit_v(1)
    emit_outproj(0)
    # h3 fully pre-computed -> its PV runs with no exp dependency; putting it
    # before h2 (whose tail exps run concurrently) shortens the critical tail
    emit_pv_norm(1, [0, 1, 3, 2])
    emit_outproj(1)


def build_nc():
    from contextlib import ExitStack

    nc = bass.Bass()
    # x for q/k: fp8 in DoubleRow d-pair layout, chunked by 512 s-columns:
    # [n, p, c, b, s] with d = c*128 + b*64 + p (64 partitions, base 0)
    xq8 = nc.dram_tensor("xq8", [S // 512, 64, DC, 2, 512], F8,
                         kind="ExternalInput")[:]
    xk8 = nc.dram_tensor("xk8", [S // 512, 64, DC, 2, 512], F8,
                         kind="ExternalInput")[:]
    # x for v: fp16 [n, p, dc, s] with d = dc*128 + p
    xv = nc.dram_tensor("xv", [S // 512, P, DC, 512], F16,
                        kind="ExternalInput")[:]
    # w for q/k: fp8 pre-scaled by WSC, same d-pair layout
    wq8 = nc.dram_tensor("wq8", [64, DC, 2, E], F8,
                         kind="ExternalInput")[:]
    wk8 = nc.dram_tensor("wk8", [64, DC, 2, E], F8,
                         kind="ExternalInput")[:]
    wv = nc.dram_tensor("wv", [P, DC, E], F16, kind="ExternalInput")[:]
    wo = nc.dram_tensor("wo", [P, EC, D], F16, kind="ExternalInput")[:]
    bq = nc.dram_tensor("bq", [E], F32, kind="ExternalInput")[:]
    bk = nc.dram_tensor("bk", [E], F32, kind="ExternalInput")[:]
    bv = nc.dram_tensor("bv", [E], F32, kind="ExternalInput")[:]
    tri = nc.dram_tensor("tri", [P, P], F16, kind="ExternalInput")[:]
    out = nc.dram_tensor("out", [S, D], F16, kind="ExternalOutput")[:]
    with tile.TileContext(nc) as tc:
        with ExitStack() as ctx:
            _emit(ctx, tc, xq8, xk8, xv, wq8, wk8, wv, wo, bq, bk, bv, tri,
                  out)
    return nc


_CACHE = {}


def _get_nc():
    if "nc" not in _CACHE:
        _CACHE["nc"] = build_nc()
    return _CACHE["nc"]


def make_in_maps(query_input, key_input, value_input, W_Q, W_K, W_V, W_O,
                 b_Q, b_K, b_V, b_O):
    qi = np.asarray(query_input, dtype=np.float32)
    ki = np.asarray(key_input, dtype=np.float32)
    vi = np.asarray(value_input, dtype=np.float32)
    W_Q = np.asarray(W_Q, dtype=np.float32)
    W_K = np.asarray(W_K, dtype=np.float32)
    W_V = np.asarray(W_V, dtype=np.float32)
    W_O = np.asarray(W_O, dtype=np.float32)
    b_Q = np.asarray(b_Q, dtype=np.float32)
    b_K = np.asarray(b_K, dtype=np.float32)
    b_V = np.asarray(b_V, dtype=np.float32)

    tri128 = np.triu(np.ones((P, P), dtype=np.float16))  # tri[j, i] = i >= j

    def pair_x2(xT8):
        # xT8: [D, S] fp8 -> [n, p, c, b, 512] with d = c*128 + b*64 + p
        a = xT8.reshape(DC, 2, 64, S // 512, 512)  # c b p n s
        a = a.transpose(3, 2, 0, 1, 4)             # n p c b s
        return np.ascontiguousarray(a)

    def pair_w(w8):
        # w8: [D, E] fp8 -> [p, c, b, E] with d = c*128 + b*64 + p
        a = w8.reshape(DC, 2, 64, E)  # c b p e
        a = a.transpose(2, 0, 1, 3)   # p c b e
        return np.ascontiguousarray(a)

    xT8 = {}
    xTv = {}
    for b in range(B):
        xT8[("q", b)] = pair_x2(
            np.ascontiguousarray(qi[b].T).astype(NP8))
        xT8[("k", b)] = pair_x2(
            np.ascontiguousarray(ki[b].T).astype(NP8))
        # v path: [n, p, dc, 512] fp16 with d = dc*128 + p
        a = np.ascontiguousarray(vi[b].T).astype(np.float16)
        a = a.reshape(DC, P, S // 512, 512).transpose(2, 1, 0, 3)
        xTv[b] = np.ascontiguousarray(a)

    in_maps = []
    for core in range(NCORES):
        b, hg = core // (NCORES // B), core % (NCORES // B)
        hs = slice(hg * HL, (hg + 1) * HL)
        wq_flat = np.transpose(W_Q[hs], (1, 0, 2)).reshape(D, E)
        wk_flat = np.transpose(W_K[hs], (1, 0, 2)).reshape(D, E)
        wv_flat = np.transpose(W_V[hs], (1, 0, 2)).reshape(D, E)
        in_maps.append({
            "xq8": xT8[("q", b)],
            "xk8": xT8[("k", b)],
            "xv": xTv[b],
            "wq8": pair_w((wq_flat * WSC).astype(NP8)),
            "wk8": pair_w((wk_flat * WSC).astype(NP8)),
            "wv": np.ascontiguousarray(
                wv_flat.reshape(DC, P, E).transpose(1, 0, 2)).astype(
                    np.float16),
            "wo": np.ascontiguousarray(
                W_O[hs].reshape(E, D).reshape(EC, P, D).transpose(
                    1, 0, 2)).astype(np.float16),
            "bq": np.ascontiguousarray(
                (b_Q[hs].reshape(E) * WSC).astype(np.float32)),
            "bk": np.ascontiguousarray(
                (b_K[hs].reshape(E) * WSC).astype(np.float32)),
            "bv": np.ascontiguousarray(b_V[hs].reshape(E)),
            "tri": tri128,
        })
    return in_maps


def gather_out(results, b_O):
    out = np.zeros((B, S, D), dtype=np.float64)
    for core in range(NCORES):
        out[core // (NCORES // B)] += results[core]["out"].astype(np.float64)
    out += np.asarray(b_O, dtype=np.float64)
    return out.astype(np.float32)


def kernel(query_input, key_input, value_input, W_Q, W_K, W_V, W_O,
           b_Q, b_K, b_V, b_O):
    nc = _get_nc()
    in_maps = make_in_maps(query_input, key_input, value_input,
                           W_Q, W_K, W_V, W_O, b_Q, b_K, b_V, b_O)
    res = run_bass_kernel_spmd(nc, in_maps, list(range(NCORES)))
    return gather_out(res.results, b_O)


def kernel_timed(inputs, trace_cores=None, **kwargs):
    """Like kernel() but traces and returns (out, BassKernelResults)."""
    nc = _get_nc()
    in_maps = make_in_maps(**inputs)
    res = run_bass_kernel_spmd(
        nc, in_maps, list(range(NCORES)), trace=True,
        trace_cores=trace_cores, **kwargs)
    return gather_out(res.results, inputs["b_O"]), res
